# revision 44
# baseline (speedup 1.0000x reference)
"""Trainium2 Bass kernel for nn_Caps_Layer (capsule routing layer).

Reference computation (per batch b of 1024):
  u_hat[b] = (x[b] @ W).reshape(512, 5, 5)
  4 rounds of routing:
    c = softmax_over_cap(blog); o = squash(sum_s c*u); blog = einsum(o, u)
  output: o [1024, 5, 5]

Sharding: pure data parallel over batch across 8 cores (128 batches/core).

Per-core design (token-position on SBUF partitions; s = 4p + q):
  - x streamed per 4-batch quad into [128p, 4*4*120] f32 (large contiguous
    DMAs keep the shared HWDGE descriptor generator off the critical path).
  - bf16 transposes via a strided 16-bit view of the f32 data (the high
    half-word of an f32 IS its bf16 truncation): 1 cyc/row on PE, no
    conversion pass, and the PSUM->SBUF xt copy runs in 2x 16-bit mode.
  - bf16 GEMM vs W; matmuls write PSUM with a strided 2-dim out AP so a
    quad lands as (k,i,b4,q4) -- the quad scatter to u2 (k,i,b,q) then has
    (b,q) contiguous and collapses to a 3-dim AP (ACT-legal, one copy).
  - Routing on-chip in fp16, layout (k,i,b,q): sum_s via PE ones-matmul,
    uo product per-chunk on GPSIMD (otherwise idle), k/i reductions as
    paired plane adds, r = 1/||o_raw|| via an int16-bit-trick rsqrt seed
    through ACT Exp (keeps a single activation-table set; a DVE Newton
    step refines the last iteration's r, which scales the output).
  - UNEVEN batch groups [48, 36, 24, 12, 8] pipeline phase1 vs routing:
    later groups (whose u_hat is DMA-gated until late) have shorter
    routing chains, balancing all groups' finish times.
"""

import numpy as np

NCORES = 8
B, S, D = 1024, 512, 120
NCAP, DCAP = 5, 5
IK = NCAP * DCAP  # 25
BC = B // NCORES  # 128 batches per core
TOK = BC * S
ROUTINGS = 4
QB = 4              # s-phases per partition (s = 4p + q)

GSIZES = [48, 36, 24, 12, 8]    # batches per group (sum = BC)
GCHUNK = [12, 12, 12, 12, 8]    # routing psum chunk (<=20 => <=1 psum bank)
NG = len(GSIZES)

N_XT_DVE = 2  # of every 8 xt copies, how many go to DVE (rest ACT)


CURRENT_LABEL = [""]


class _Geo:
    """Per-group layout geometry. u2/cu free dims (k5, i5, bG, q4)."""

    def __init__(self, g):
        self.bg = GSIZES[g]                 # batches in group
        self.b0 = sum(GSIZES[:g])           # first batch
        self.chunk = GCHUNK[g]
        self.nchunk = self.bg // self.chunk
        self.nquad = self.bg // 4
        self.q0 = self.b0 // 4              # first quad (absolute)
        self.k_str = NCAP * self.bg * QB
        self.i_str = self.bg * QB
        self.fg = DCAP * self.k_str
        self.ok = NCAP * self.bg            # o2 k stride
        self.oi = self.bg                   # o2 i stride
        self.pl = NCAP * self.bg            # (i,b) plane
        self.blf = NCAP * self.bg * QB      # (i,b,q) logits
        self.zf = self.bg * QB              # (b,q)


def _build(n_routing=ROUTINGS):
    import math

    import concourse.bass as bass
    import concourse.bacc as bacc
    import concourse.tile as tile
    from concourse import mybir
    from concourse.masks import make_identity

    f32 = mybir.dt.float32
    f16 = mybir.dt.float16
    bf16 = mybir.dt.bfloat16
    i16 = mybir.dt.int16
    AF = mybir.ActivationFunctionType
    # rsqrt-from-f16-bits: int16 view of positive f16 x is affine in
    # log2(x), so r0 = exp(S*bits + C0) ~= x**-0.5 (max rel err ~1.5%);
    # Exp is in the already-loaded ACT table set -> no table switches.
    RS_S = -0.5 * math.log(2.0) / 1024.0
    RS_C0 = 0.5 * math.log(2.0) * (15.0 - 0.043)

    geos = [_Geo(g) for g in range(NG)]

    nc = bacc.Bacc("TRN2", target_bir_lowering=False, debug=False)
    x_d = nc.dram_tensor("x", [TOK, D], f32, kind="ExternalInput")
    w_d = nc.dram_tensor("w", [D, IK], f32, kind="ExternalInput")
    out_d = nc.dram_tensor("out", [1, BC * IK], f32, kind="ExternalOutput")

    # HBM elem(quad c; f, p, q, d) = c*4*61440 + f*61440 + p*480 + q*120 + d
    xr = x_d[:, :]

    def xv4(c):
        return bass.AP(
            tensor=xr.tensor,
            offset=xr.offset + c * 4 * 128 * QB * D,
            ap=[[QB * D, 128], [128 * QB * D, 4], [1, QB * D]],
        )

    def ap_of(tile_ap, free_dims, extra_off=0):
        return bass.AP(
            tensor=tile_ap.tensor,
            offset=tile_ap.offset + extra_off,
            ap=[list(tile_ap.ap[0])] + [list(d) for d in free_dims],
        )

    def row0(tile_ap, free_dims, extra_off=0):
        p0 = [list(tile_ap.ap[0])[0], 1]
        return bass.AP(
            tensor=tile_ap.tensor,
            offset=tile_ap.offset + extra_off,
            ap=[p0] + [list(d) for d in free_dims],
        )

    with tile.TileContext(nc) as tc:
        with (
            tc.tile_pool(name="const", bufs=1) as const,
            tc.tile_pool(name="big", bufs=1) as big,
            tc.tile_pool(name="xin", bufs=4) as xin,
            tc.tile_pool(name="xtsb", bufs=4) as xtsb,
            tc.tile_pool(name="xtps", bufs=2, space="PSUM") as xtps,
            tc.tile_pool(name="ups", bufs=2, space="PSUM") as ups,
            tc.tile_pool(name="ops", bufs=4, space="PSUM") as ops_pool,
        ):
            # ---- constants ----
            w_sb = const.tile([128, IK], f32)
            nc.sync.dma_start(out=w_sb[:D, :], in_=w_d[:, :])
            w16 = const.tile([128, IK], bf16)
            nc.vector.tensor_copy(out=w16[:D, :], in_=w_sb[:D, :])
            ident16 = const.tile([128, 128], bf16)
            make_identity(nc, ident16[:])
            ones16 = const.tile([128, 128], f16)
            nc.vector.memset(ones16[:], 1.0)
            c0t = const.tile([128, 1], f32)
            nc.vector.memset(c0t[:], RS_C0)

            # ---- per-group persistent tensors ----
            u2, cu, o2, sq, bl, pb, e_t, c_t = [], [], [], [], [], [], [], []
            zp, z_t, zi, sp, ss, lr, rr, fin = [], [], [], [], [], [], [], []
            for g, geo in enumerate(geos):
                u2.append(big.tile([128, geo.fg], f16, name=f"u2_{g}"))
                cu.append(big.tile([128, geo.fg], f16, name=f"cu_{g}"))
                o2.append(big.tile([128, DCAP * geo.pl], f16, name=f"o2_{g}"))
                sq.append(big.tile([128, DCAP * geo.pl], f16, name=f"sq_{g}"))
                bl.append(big.tile([128, geo.blf], f16, name=f"bl_{g}"))
                pb.append(big.tile([128, 2 * geo.blf], f16, name=f"pb_{g}"))
                e_t.append(big.tile([128, geo.blf], f16, name=f"e_{g}"))
                c_t.append(big.tile([128, geo.blf], f16, name=f"c_{g}"))
                zp.append(big.tile([128, 2 * geo.zf], f16, name=f"zp_{g}"))
                z_t.append(big.tile([128, geo.zf], f16, name=f"z_{g}"))
                zi.append(big.tile([128, geo.zf], f16, name=f"zi_{g}"))
                sp.append(big.tile([128, 2 * geo.pl], f16, name=f"sp_{g}"))
                ss.append(big.tile([128, geo.pl], f16, name=f"ss_{g}"))
                lr.append(big.tile([128, geo.pl], f32, name=f"lr_{g}"))
                rr.append(big.tile([128, geo.pl], f32, name=f"rr_{g}"))
                fin.append(big.tile([1, geo.bg * IK], f32, name=f"fin_{g}"))

            # ================= Phase 1: u_hat GEMM =================
            def phase1_quad(g, gl):
                CURRENT_LABEL[0] = f"p1.g{g}"
                geo = geos[g]
                if True:
                    # psum quad laid out (k5, i5, b4, q4): strides 80,16,4,1
                    u_ps = ups.tile([128, 16 * IK], f32, name="u_ps")
                    x_sb = xin.tile([128, 4 * QB * D], f32, name="x_sb")
                    nc.sync.dma_start(out=x_sb[:], in_=xv4(geo.q0 + gl))
                    a16 = x_sb[:].bitcast(bf16)
                    for hh in range(2):
                        xt_ps = xtps.tile([128, 1024], bf16, name="xt_ps")
                        for bb in range(2):
                            f = hh * 2 + bb
                            for q in range(QB):
                                t = bb * QB + q
                                src = bass.AP(
                                    tensor=a16.tensor,
                                    offset=a16.offset
                                    + 2 * (f * QB * D + q * D) + 1,
                                    ap=[list(a16.ap[0]), [2, D]],
                                )
                                nc.tensor.transpose(
                                    xt_ps[:D, t * 128:(t + 1) * 128],
                                    src, ident16[:],
                                )
                        xt_sb = xtsb.tile([128, 1024], bf16, name="xt_sb")
                        if (gl * 2 + hh) % 8 < N_XT_DVE:
                            nc.vector.tensor_copy(
                                out=xt_sb[:D, :], in_=xt_ps[:D, :])
                        else:
                            nc.scalar.copy(out=xt_sb[:D, :], in_=xt_ps[:D, :])
                        for t in range(8):
                            bb_, q_ = t // QB, t % QB
                            bloc = hh * 2 + bb_
                            # out cols (i,k) -> psum (k:80, i:16) + b*4 + q
                            dst = ap_of(u_ps[:], [[16, NCAP], [80, DCAP]],
                                        extra_off=bloc * 4 + q_)
                            nc.tensor.matmul(
                                dst,
                                xt_sb[:D, t * 128:(t + 1) * 128],
                                w16[:D, :],
                                start=True, stop=True,
                            )
                    # quad scatter (one 3-dim copy): psum (k,i,b4,q4) ->
                    # u2 (k,i,b,q); (b4,q4) is a contiguous run of 16 both
                    # sides.
                    src = ap_of(u_ps[:], [[80, DCAP], [16, NCAP], [1, 16]])
                    dst = ap_of(u2[g][:], [[geo.k_str, DCAP],
                                           [geo.i_str, NCAP], [1, 16]],
                                extra_off=gl * 4 * QB)
                    nc.scalar.copy(out=dst, in_=src)

            # ================= Phase 2: routing =================
            # Issue stages batched across groups so no engine's in-order
            # queue head-of-line blocks on another group's dependency.
            def routing_stage_a(g, it):
                CURRENT_LABEL[0] = f"a.g{g}.i{it}"
                geo = geos[g]
                u2g, cug, o2g = u2[g][:], cu[g][:], o2[g][:]
                last = it == n_routing - 1
                src_t = u2g if it == 0 else cug
                ck = geo.chunk
                for ci in range(geo.nchunk):
                    o_ps = ops_pool.tile([128, ck * IK], f32, name="o_ps")
                    for q in range(QB):
                        rhs = ap_of(
                            src_t,
                            [[QB, ck], [geo.i_str, NCAP], [geo.k_str, DCAP]],
                            extra_off=q + ci * ck * QB,
                        )
                        nc.tensor.matmul(
                            o_ps[:], ones16[:], rhs,
                            start=(q == 0), stop=(q == QB - 1),
                        )
                    # psum (b,i,k) -> o2 (k,i,b), cast f16
                    dst = ap_of(
                        o2g, [[1, ck], [geo.oi, NCAP], [geo.ok, DCAP]],
                        extra_off=ci * ck,
                    )
                    nc.scalar.copy(out=dst, in_=o_ps[:])
                    if not last:
                        # uo chunk on GPSIMD: cu = u2 * o2_bcast(q)
                        u2c = ap_of(u2g, [[geo.k_str, DCAP],
                                          [geo.i_str, NCAP], [1, ck * QB]],
                                    extra_off=ci * ck * QB)
                        cuc = ap_of(cug, [[geo.k_str, DCAP],
                                          [geo.i_str, NCAP], [1, ck * QB]],
                                    extra_off=ci * ck * QB)
                        o2_bc = ap_of(o2g, [[geo.ok, DCAP], [geo.oi, NCAP],
                                            [1, ck], [0, QB]],
                                      extra_off=ci * ck)
                        nc.gpsimd.tensor_mul(cuc, u2c, o2_bc)

            def routing_stats(g, it):
                CURRENT_LABEL[0] = f"s.g{g}.i{it}"
                geo = geos[g]
                o2g, sqg = o2[g][:], sq[g][:]
                last = it == n_routing - 1
                pl = geo.pl
                # squash stats: ss = sum_k o^2 -> rr = 1/sqrt(ss)
                spg = sp[g][:]
                nc.vector.tensor_mul(sqg, o2g, o2g)
                nc.vector.tensor_add(
                    spg,
                    ap_of(sqg, [[2 * pl, 2], [1, pl]]),
                    ap_of(sqg, [[2 * pl, 2], [1, pl]], extra_off=pl))
                nc.vector.tensor_add(ss[g][:], sp[g][:, :pl], sp[g][:, pl:])
                nc.vector.tensor_add(
                    ss[g][:], ss[g][:],
                    ap_of(sqg, [[1, pl]], extra_off=4 * pl))
                nc.scalar.activation(
                    out=rr[g][:], in_=ss[g][:].bitcast(i16), func=AF.Exp,
                    scale=RS_S, bias=c0t[:])
                if last:
                    # one Newton step: r *= 1.5 - 0.5*ss*r^2 (max err 6e-4);
                    # only the last iteration's r scales the output directly
                    nt = lr[g][:]
                    nc.vector.tensor_mul(nt, rr[g][:], rr[g][:])
                    nc.vector.tensor_mul(nt, nt, ss[g][:])
                    nc.vector.tensor_scalar(
                        out=nt, in0=nt, scalar1=-0.5, scalar2=1.5,
                        op0=mybir.AluOpType.mult, op1=mybir.AluOpType.add)
                    nc.vector.tensor_mul(rr[g][:], rr[g][:], nt)

            def routing_stage_b(g, it):
                CURRENT_LABEL[0] = f"b.g{g}.i{it}"
                geo = geos[g]
                u2g, cug, o2g = u2[g][:], cu[g][:], o2[g][:]
                last = it == n_routing - 1
                blf, zf, pl = geo.blf, geo.zf, geo.pl
                if not last:
                    # blog = sum_k uo (k-plane pair adds), fold r, softmax(i)
                    blg, pbg = bl[g][:], pb[g][:]
                    ks = geo.k_str
                    nc.vector.tensor_add(
                        pbg,
                        ap_of(cug, [[2 * ks, 2], [1, blf]]),
                        ap_of(cug, [[2 * ks, 2], [1, blf]], extra_off=ks))
                    nc.vector.tensor_add(blg, pb[g][:, :blf], pb[g][:, blf:])
                    nc.vector.tensor_add(
                        blg, blg, ap_of(cug, [[1, blf]], extra_off=4 * ks))
                    # r broadcast (i,b) -> (i,b,q): strided, 1x
                    r32_bc = ap_of(rr[g][:], [[geo.oi, NCAP], [1, geo.bg],
                                              [0, QB]])
                    nc.vector.tensor_mul(blg, blg, r32_bc)
                    nc.scalar.activation(out=e_t[g][:], in_=blg, func=AF.Exp)
                    # z = sum_i e  (i-planes of (i,b,q))
                    nc.vector.tensor_add(
                        zp[g][:],
                        ap_of(e_t[g][:], [[2 * zf, 2], [1, zf]]),
                        ap_of(e_t[g][:], [[2 * zf, 2], [1, zf]],
                              extra_off=zf))
                    nc.vector.tensor_add(z_t[g][:], zp[g][:, :zf],
                                         zp[g][:, zf:])
                    nc.vector.tensor_add(
                        z_t[g][:], z_t[g][:],
                        ap_of(e_t[g][:], [[1, zf]], extra_off=4 * zf))
                    with nc.allow_low_precision("softmax denom fp16 ok"):
                        nc.vector.reciprocal(zi[g][:], z_t[g][:])
                    zi_bc = ap_of(zi[g][:], [[0, NCAP], [1, zf]])
                    nc.vector.tensor_mul(c_t[g][:], e_t[g][:], zi_bc)
                    # cu = u2 * c_bcast(k), per chunk so the next
                    # iteration's first matmul can start early
                    ck = geo.chunk
                    for ci in range(geo.nchunk):
                        u2c = ap_of(u2g, [[geo.k_str, DCAP],
                                          [geo.i_str, NCAP], [1, ck * QB]],
                                    extra_off=ci * ck * QB)
                        cuc = ap_of(cug, [[geo.k_str, DCAP],
                                          [geo.i_str, NCAP], [1, ck * QB]],
                                    extra_off=ci * ck * QB)
                        c_bc = ap_of(c_t[g][:],
                                     [[0, DCAP], [geo.i_str, NCAP],
                                      [1, ck * QB]],
                                     extra_off=ci * ck * QB)
                        nc.vector.tensor_mul(cuc, u2c, c_bc)
                else:
                    # fin[(b,i,k)] = o2[(k,i,b)] * r  (row 0; all rows equal)
                    o2_row = row0(o2g, [[1, geo.bg], [geo.oi, NCAP],
                                        [geo.ok, DCAP]])
                    r_row = row0(rr[g][:], [[1, geo.bg], [geo.bg, NCAP],
                                            [0, DCAP]])
                    nc.gpsimd.tensor_mul(fin[g][:], o2_row, r_row)
                    nc.sync.dma_start(
                        out=out_d[:, geo.b0 * IK:(geo.b0 + geo.bg) * IK],
                        in_=fin[g][:],
                    )

            # Feasibility-ordered global issue: each engine's in-order queue
            # then approximates the true dependency order, minimizing
            # head-of-line blocking. Keys are rough start-time estimates
            # (us): DMA delivers ~0.683us/batch; a routing iteration's
            # serial chain is ~(chain_a + chain_b) us.
            units = []
            for g, geo in enumerate(geos):
                for gl in range(geo.nquad):
                    units.append((0.683 * 4 * (geo.q0 + gl), 0,
                                  ("p1", g, gl)))
                p1_end = 0.683 * (geo.b0 + geo.bg) + 1.5
                chain_a = 1.5 + 0.21 * geo.bg   # matmul+o2+uo
                chain_b = 4.0 + 0.09 * geo.bg   # blog+softmax+cu
                tkey = p1_end
                for it in range(n_routing):
                    units.append((tkey, 1, ("a", g, it)))
                    units.append((tkey + 1.5, 2, ("s", g, it)))
                    units.append((tkey + chain_a, 3, ("b", g, it)))
                    tkey += chain_a + chain_b
            units.sort(key=lambda u: (u[0], u[1]))
            for _, _, (kind, g, x) in units:
                if kind == "p1":
                    phase1_quad(g, x)
                elif kind == "a":
                    routing_stage_a(g, x)
                elif kind == "s":
                    routing_stats(g, x)
                else:
                    routing_stage_b(g, x)
    nc.compile()
    return nc


_NC = None


def kernel(x: np.ndarray, W: np.ndarray) -> np.ndarray:
    from concourse.bass_utils import run_bass_kernel_spmd

    global _NC
    if _NC is None:
        _NC = _build()

    x = np.ascontiguousarray(x, dtype=np.float32)
    w = np.ascontiguousarray(W.reshape(D, IK), dtype=np.float32)
    xs = x.reshape(NCORES, TOK, D)
    in_maps = [{"x": xs[i], "w": w} for i in range(NCORES)]
    res = run_bass_kernel_spmd(_NC, in_maps, core_ids=list(range(NCORES)))
    out = np.concatenate(
        [r["out"].reshape(BC, NCAP, DCAP) for r in res.results], axis=0
    )
    return out


if __name__ == "__main__":
    rng = np.random.default_rng(0)
    x = rng.standard_normal((B, S, D), dtype=np.float32)
    W = rng.standard_normal((1, D, IK), dtype=np.float32) * 0.1
    out = kernel(x, W)
    print(out.shape, out.dtype)


# revision 45
# speedup vs baseline: 1.0110x; 1.0110x over previous
"""Trainium2 Bass kernel for nn_Caps_Layer (capsule routing layer).

Reference computation (per batch b of 1024):
  u_hat[b] = (x[b] @ W).reshape(512, 5, 5)
  4 rounds of routing:
    c = softmax_over_cap(blog); o = squash(sum_s c*u); blog = einsum(o, u)
  output: o [1024, 5, 5]

Sharding: pure data parallel over batch across 8 cores (128 batches/core).

Per-core design (token-position on SBUF partitions; s = 4p + q):
  - x streamed per 4-batch quad into [128p, 4*4*120] f32 (large contiguous
    DMAs keep the shared HWDGE descriptor generator off the critical path).
  - bf16 transposes via a strided 16-bit view of the f32 data (the high
    half-word of an f32 IS its bf16 truncation): 1 cyc/row on PE, no
    conversion pass, and the PSUM->SBUF xt copy runs in 2x 16-bit mode.
  - bf16 GEMM vs W; matmuls write PSUM with a strided 2-dim out AP so a
    quad lands as (k,i,b4,q4) -- the quad scatter to u2 (k,i,b,q) then has
    (b,q) contiguous and collapses to a 3-dim AP (ACT-legal, one copy).
  - Routing on-chip in fp16, layout (k,i,b,q): sum_s via PE ones-matmul,
    uo product per-chunk on GPSIMD (otherwise idle), k/i reductions as
    paired plane adds, r = 1/||o_raw|| via an int16-bit-trick rsqrt seed
    through ACT Exp (keeps a single activation-table set; a DVE Newton
    step refines the last iteration's r, which scales the output).
  - UNEVEN batch groups [48, 36, 24, 12, 8] pipeline phase1 vs routing:
    later groups (whose u_hat is DMA-gated until late) have shorter
    routing chains, balancing all groups' finish times.
"""

import numpy as np

NCORES = 8
B, S, D = 1024, 512, 120
NCAP, DCAP = 5, 5
IK = NCAP * DCAP  # 25
BC = B // NCORES  # 128 batches per core
TOK = BC * S
ROUTINGS = 4
QB = 4              # s-phases per partition (s = 4p + q)

GSIZES = [48, 36, 24, 12, 8]    # batches per group (sum = BC)
GCHUNK = [16, 18, 12, 12, 8]    # routing psum chunk (<=20 => <=1 psum bank)
NG = len(GSIZES)

N_XT_DVE = 2  # of every 8 xt copies, how many go to DVE (rest ACT)


CURRENT_LABEL = [""]


class _Geo:
    """Per-group layout geometry. u2/cu free dims (k5, i5, bG, q4)."""

    def __init__(self, g):
        self.bg = GSIZES[g]                 # batches in group
        self.b0 = sum(GSIZES[:g])           # first batch
        self.chunk = GCHUNK[g]
        self.nchunk = self.bg // self.chunk
        self.nquad = self.bg // 4
        self.q0 = self.b0 // 4              # first quad (absolute)
        self.k_str = NCAP * self.bg * QB
        self.i_str = self.bg * QB
        self.fg = DCAP * self.k_str
        self.ok = NCAP * self.bg            # o2 k stride
        self.oi = self.bg                   # o2 i stride
        self.pl = NCAP * self.bg            # (i,b) plane
        self.blf = NCAP * self.bg * QB      # (i,b,q) logits
        self.zf = self.bg * QB              # (b,q)


def _build(n_routing=ROUTINGS):
    import math

    import concourse.bass as bass
    import concourse.bacc as bacc
    import concourse.tile as tile
    from concourse import mybir
    from concourse.masks import make_identity

    f32 = mybir.dt.float32
    f16 = mybir.dt.float16
    bf16 = mybir.dt.bfloat16
    i16 = mybir.dt.int16
    AF = mybir.ActivationFunctionType
    # rsqrt-from-f16-bits: int16 view of positive f16 x is affine in
    # log2(x), so r0 = exp(S*bits + C0) ~= x**-0.5 (max rel err ~1.5%);
    # Exp is in the already-loaded ACT table set -> no table switches.
    RS_S = -0.5 * math.log(2.0) / 1024.0
    RS_C0 = 0.5 * math.log(2.0) * (15.0 - 0.043)

    geos = [_Geo(g) for g in range(NG)]

    nc = bacc.Bacc("TRN2", target_bir_lowering=False, debug=False)
    x_d = nc.dram_tensor("x", [TOK, D], f32, kind="ExternalInput")
    w_d = nc.dram_tensor("w", [D, IK], f32, kind="ExternalInput")
    out_d = nc.dram_tensor("out", [1, BC * IK], f32, kind="ExternalOutput")

    # HBM elem(quad c; f, p, q, d) = c*4*61440 + f*61440 + p*480 + q*120 + d
    xr = x_d[:, :]

    def xv4(c):
        return bass.AP(
            tensor=xr.tensor,
            offset=xr.offset + c * 4 * 128 * QB * D,
            ap=[[QB * D, 128], [128 * QB * D, 4], [1, QB * D]],
        )

    def ap_of(tile_ap, free_dims, extra_off=0):
        return bass.AP(
            tensor=tile_ap.tensor,
            offset=tile_ap.offset + extra_off,
            ap=[list(tile_ap.ap[0])] + [list(d) for d in free_dims],
        )

    def row0(tile_ap, free_dims, extra_off=0):
        p0 = [list(tile_ap.ap[0])[0], 1]
        return bass.AP(
            tensor=tile_ap.tensor,
            offset=tile_ap.offset + extra_off,
            ap=[p0] + [list(d) for d in free_dims],
        )

    with tile.TileContext(nc) as tc:
        with (
            tc.tile_pool(name="const", bufs=1) as const,
            tc.tile_pool(name="big", bufs=1) as big,
            tc.tile_pool(name="xin", bufs=4) as xin,
            tc.tile_pool(name="xtsb", bufs=4) as xtsb,
            tc.tile_pool(name="xtps", bufs=2, space="PSUM") as xtps,
            tc.tile_pool(name="ups", bufs=2, space="PSUM") as ups,
            tc.tile_pool(name="ops", bufs=4, space="PSUM") as ops_pool,
        ):
            # ---- constants ----
            w_sb = const.tile([128, IK], f32)
            nc.sync.dma_start(out=w_sb[:D, :], in_=w_d[:, :])
            w16 = const.tile([128, IK], bf16)
            nc.vector.tensor_copy(out=w16[:D, :], in_=w_sb[:D, :])
            ident16 = const.tile([128, 128], bf16)
            make_identity(nc, ident16[:])
            ones16 = const.tile([128, 128], f16)
            nc.vector.memset(ones16[:], 1.0)
            c0t = const.tile([128, 1], f32)
            nc.vector.memset(c0t[:], RS_C0)

            # ---- per-group persistent tensors ----
            u2, cu, o2, sq, bl, pb, e_t, c_t = [], [], [], [], [], [], [], []
            zp, z_t, zi, sp, ss, lr, rr, fin = [], [], [], [], [], [], [], []
            for g, geo in enumerate(geos):
                u2.append(big.tile([128, geo.fg], f16, name=f"u2_{g}"))
                cu.append(big.tile([128, geo.fg], f16, name=f"cu_{g}"))
                o2.append(big.tile([128, DCAP * geo.pl], f16, name=f"o2_{g}"))
                sq.append(big.tile([128, DCAP * geo.pl], f16, name=f"sq_{g}"))
                bl.append(big.tile([128, geo.blf], f16, name=f"bl_{g}"))
                pb.append(big.tile([128, 2 * geo.blf], f16, name=f"pb_{g}"))
                e_t.append(big.tile([128, geo.blf], f16, name=f"e_{g}"))
                c_t.append(big.tile([128, geo.blf], f16, name=f"c_{g}"))
                zp.append(big.tile([128, 2 * geo.zf], f16, name=f"zp_{g}"))
                z_t.append(big.tile([128, geo.zf], f16, name=f"z_{g}"))
                zi.append(big.tile([128, geo.zf], f16, name=f"zi_{g}"))
                sp.append(big.tile([128, 2 * geo.pl], f16, name=f"sp_{g}"))
                ss.append(big.tile([128, geo.pl], f16, name=f"ss_{g}"))
                lr.append(big.tile([128, geo.pl], f32, name=f"lr_{g}"))
                rr.append(big.tile([128, geo.pl], f32, name=f"rr_{g}"))
                fin.append(big.tile([1, geo.bg * IK], f32, name=f"fin_{g}"))

            # ================= Phase 1: u_hat GEMM =================
            def phase1_quad(g, gl):
                CURRENT_LABEL[0] = f"p1.g{g}"
                geo = geos[g]
                if True:
                    # psum quad laid out (k5, i5, b4, q4): strides 80,16,4,1
                    u_ps = ups.tile([128, 16 * IK], f32, name="u_ps")
                    x_sb = xin.tile([128, 4 * QB * D], f32, name="x_sb")
                    nc.sync.dma_start(out=x_sb[:], in_=xv4(geo.q0 + gl))
                    a16 = x_sb[:].bitcast(bf16)
                    for hh in range(2):
                        xt_ps = xtps.tile([128, 1024], bf16, name="xt_ps")
                        for bb in range(2):
                            f = hh * 2 + bb
                            for q in range(QB):
                                t = bb * QB + q
                                src = bass.AP(
                                    tensor=a16.tensor,
                                    offset=a16.offset
                                    + 2 * (f * QB * D + q * D) + 1,
                                    ap=[list(a16.ap[0]), [2, D]],
                                )
                                nc.tensor.transpose(
                                    xt_ps[:D, t * 128:(t + 1) * 128],
                                    src, ident16[:],
                                )
                        xt_sb = xtsb.tile([128, 1024], bf16, name="xt_sb")
                        if (gl * 2 + hh) % 8 < N_XT_DVE:
                            nc.vector.tensor_copy(
                                out=xt_sb[:D, :], in_=xt_ps[:D, :])
                        else:
                            nc.scalar.copy(out=xt_sb[:D, :], in_=xt_ps[:D, :])
                        for t in range(8):
                            bb_, q_ = t // QB, t % QB
                            bloc = hh * 2 + bb_
                            # out cols (i,k) -> psum (k:80, i:16) + b*4 + q
                            dst = ap_of(u_ps[:], [[16, NCAP], [80, DCAP]],
                                        extra_off=bloc * 4 + q_)
                            nc.tensor.matmul(
                                dst,
                                xt_sb[:D, t * 128:(t + 1) * 128],
                                w16[:D, :],
                                start=True, stop=True,
                            )
                    # quad scatter (one 3-dim copy): psum (k,i,b4,q4) ->
                    # u2 (k,i,b,q); (b4,q4) is a contiguous run of 16 both
                    # sides.
                    src = ap_of(u_ps[:], [[80, DCAP], [16, NCAP], [1, 16]])
                    dst = ap_of(u2[g][:], [[geo.k_str, DCAP],
                                           [geo.i_str, NCAP], [1, 16]],
                                extra_off=gl * 4 * QB)
                    nc.scalar.copy(out=dst, in_=src)

            # ================= Phase 2: routing =================
            # Issue stages batched across groups so no engine's in-order
            # queue head-of-line blocks on another group's dependency.
            def routing_stage_a(g, it):
                CURRENT_LABEL[0] = f"a.g{g}.i{it}"
                geo = geos[g]
                u2g, cug, o2g = u2[g][:], cu[g][:], o2[g][:]
                last = it == n_routing - 1
                src_t = u2g if it == 0 else cug
                ck = geo.chunk
                for ci in range(geo.nchunk):
                    o_ps = ops_pool.tile([128, ck * IK], f32, name="o_ps")
                    for q in range(QB):
                        rhs = ap_of(
                            src_t,
                            [[QB, ck], [geo.i_str, NCAP], [geo.k_str, DCAP]],
                            extra_off=q + ci * ck * QB,
                        )
                        nc.tensor.matmul(
                            o_ps[:], ones16[:], rhs,
                            start=(q == 0), stop=(q == QB - 1),
                        )
                    # psum (b,i,k) -> o2 (k,i,b), cast f16
                    dst = ap_of(
                        o2g, [[1, ck], [geo.oi, NCAP], [geo.ok, DCAP]],
                        extra_off=ci * ck,
                    )
                    nc.scalar.copy(out=dst, in_=o_ps[:])
                    if not last:
                        # uo chunk on GPSIMD: cu = u2 * o2_bcast(q)
                        u2c = ap_of(u2g, [[geo.k_str, DCAP],
                                          [geo.i_str, NCAP], [1, ck * QB]],
                                    extra_off=ci * ck * QB)
                        cuc = ap_of(cug, [[geo.k_str, DCAP],
                                          [geo.i_str, NCAP], [1, ck * QB]],
                                    extra_off=ci * ck * QB)
                        o2_bc = ap_of(o2g, [[geo.ok, DCAP], [geo.oi, NCAP],
                                            [1, ck], [0, QB]],
                                      extra_off=ci * ck)
                        nc.gpsimd.tensor_mul(cuc, u2c, o2_bc)

            def routing_stats(g, it):
                CURRENT_LABEL[0] = f"s.g{g}.i{it}"
                geo = geos[g]
                o2g, sqg = o2[g][:], sq[g][:]
                last = it == n_routing - 1
                pl = geo.pl
                # squash stats: ss = sum_k o^2 -> rr = 1/sqrt(ss)
                spg = sp[g][:]
                nc.vector.tensor_mul(sqg, o2g, o2g)
                nc.vector.tensor_add(
                    spg,
                    ap_of(sqg, [[2 * pl, 2], [1, pl]]),
                    ap_of(sqg, [[2 * pl, 2], [1, pl]], extra_off=pl))
                nc.vector.tensor_add(ss[g][:], sp[g][:, :pl], sp[g][:, pl:])
                nc.vector.tensor_add(
                    ss[g][:], ss[g][:],
                    ap_of(sqg, [[1, pl]], extra_off=4 * pl))
                nc.scalar.activation(
                    out=rr[g][:], in_=ss[g][:].bitcast(i16), func=AF.Exp,
                    scale=RS_S, bias=c0t[:])
                if last:
                    # one Newton step: r *= 1.5 - 0.5*ss*r^2 (max err 6e-4);
                    # only the last iteration's r scales the output directly
                    nt = lr[g][:]
                    nc.vector.tensor_mul(nt, rr[g][:], rr[g][:])
                    nc.vector.tensor_mul(nt, nt, ss[g][:])
                    nc.vector.tensor_scalar(
                        out=nt, in0=nt, scalar1=-0.5, scalar2=1.5,
                        op0=mybir.AluOpType.mult, op1=mybir.AluOpType.add)
                    nc.vector.tensor_mul(rr[g][:], rr[g][:], nt)

            def routing_stage_b(g, it):
                CURRENT_LABEL[0] = f"b.g{g}.i{it}"
                geo = geos[g]
                u2g, cug, o2g = u2[g][:], cu[g][:], o2[g][:]
                last = it == n_routing - 1
                blf, zf, pl = geo.blf, geo.zf, geo.pl
                if not last:
                    # blog = sum_k uo (k-plane pair adds), fold r, softmax(i)
                    blg, pbg = bl[g][:], pb[g][:]
                    ks = geo.k_str
                    nc.vector.tensor_add(
                        pbg,
                        ap_of(cug, [[2 * ks, 2], [1, blf]]),
                        ap_of(cug, [[2 * ks, 2], [1, blf]], extra_off=ks))
                    nc.vector.tensor_add(blg, pb[g][:, :blf], pb[g][:, blf:])
                    nc.vector.tensor_add(
                        blg, blg, ap_of(cug, [[1, blf]], extra_off=4 * ks))
                    # r broadcast (i,b) -> (i,b,q): strided, 1x
                    r32_bc = ap_of(rr[g][:], [[geo.oi, NCAP], [1, geo.bg],
                                              [0, QB]])
                    nc.vector.tensor_mul(blg, blg, r32_bc)
                    nc.scalar.activation(out=e_t[g][:], in_=blg, func=AF.Exp)
                    # z = sum_i e  (i-planes of (i,b,q))
                    nc.vector.tensor_add(
                        zp[g][:],
                        ap_of(e_t[g][:], [[2 * zf, 2], [1, zf]]),
                        ap_of(e_t[g][:], [[2 * zf, 2], [1, zf]],
                              extra_off=zf))
                    nc.vector.tensor_add(z_t[g][:], zp[g][:, :zf],
                                         zp[g][:, zf:])
                    nc.vector.tensor_add(
                        z_t[g][:], z_t[g][:],
                        ap_of(e_t[g][:], [[1, zf]], extra_off=4 * zf))
                    with nc.allow_low_precision("softmax denom fp16 ok"):
                        nc.vector.reciprocal(zi[g][:], z_t[g][:])
                    zi_bc = ap_of(zi[g][:], [[0, NCAP], [1, zf]])
                    nc.vector.tensor_mul(c_t[g][:], e_t[g][:], zi_bc)
                    # cu = u2 * c_bcast(k), per chunk so the next
                    # iteration's first matmul can start early
                    ck = geo.chunk
                    for ci in range(geo.nchunk):
                        u2c = ap_of(u2g, [[geo.k_str, DCAP],
                                          [geo.i_str, NCAP], [1, ck * QB]],
                                    extra_off=ci * ck * QB)
                        cuc = ap_of(cug, [[geo.k_str, DCAP],
                                          [geo.i_str, NCAP], [1, ck * QB]],
                                    extra_off=ci * ck * QB)
                        c_bc = ap_of(c_t[g][:],
                                     [[0, DCAP], [geo.i_str, NCAP],
                                      [1, ck * QB]],
                                     extra_off=ci * ck * QB)
                        nc.vector.tensor_mul(cuc, u2c, c_bc)
                else:
                    # fin[(b,i,k)] = o2[(k,i,b)] * r  (row 0; all rows equal)
                    o2_row = row0(o2g, [[1, geo.bg], [geo.oi, NCAP],
                                        [geo.ok, DCAP]])
                    r_row = row0(rr[g][:], [[1, geo.bg], [geo.bg, NCAP],
                                            [0, DCAP]])
                    nc.gpsimd.tensor_mul(fin[g][:], o2_row, r_row)
                    nc.sync.dma_start(
                        out=out_d[:, geo.b0 * IK:(geo.b0 + geo.bg) * IK],
                        in_=fin[g][:],
                    )

            # Feasibility-ordered global issue: each engine's in-order queue
            # then approximates the true dependency order, minimizing
            # head-of-line blocking. Keys are rough start-time estimates
            # (us): DMA delivers ~0.683us/batch; a routing iteration's
            # serial chain is ~(chain_a + chain_b) us.
            units = []
            for g, geo in enumerate(geos):
                for gl in range(geo.nquad):
                    units.append((0.683 * 4 * (geo.q0 + gl), 0,
                                  ("p1", g, gl)))
                p1_end = 0.683 * (geo.b0 + geo.bg) + 1.5
                chain_a = 1.5 + 0.21 * geo.bg   # matmul+o2+uo
                chain_b = 4.0 + 0.09 * geo.bg   # blog+softmax+cu
                tkey = p1_end
                for it in range(n_routing):
                    units.append((tkey, 1, ("a", g, it)))
                    units.append((tkey + 1.5, 2, ("s", g, it)))
                    units.append((tkey + chain_a, 3, ("b", g, it)))
                    tkey += chain_a + chain_b
            units.sort(key=lambda u: (u[0], u[1]))
            for _, _, (kind, g, x) in units:
                if kind == "p1":
                    phase1_quad(g, x)
                elif kind == "a":
                    routing_stage_a(g, x)
                elif kind == "s":
                    routing_stats(g, x)
                else:
                    routing_stage_b(g, x)
    nc.compile()
    return nc


_NC = None


def kernel(x: np.ndarray, W: np.ndarray) -> np.ndarray:
    from concourse.bass_utils import run_bass_kernel_spmd

    global _NC
    if _NC is None:
        _NC = _build()

    x = np.ascontiguousarray(x, dtype=np.float32)
    w = np.ascontiguousarray(W.reshape(D, IK), dtype=np.float32)
    xs = x.reshape(NCORES, TOK, D)
    in_maps = [{"x": xs[i], "w": w} for i in range(NCORES)]
    res = run_bass_kernel_spmd(_NC, in_maps, core_ids=list(range(NCORES)))
    out = np.concatenate(
        [r["out"].reshape(BC, NCAP, DCAP) for r in res.results], axis=0
    )
    return out


if __name__ == "__main__":
    rng = np.random.default_rng(0)
    x = rng.standard_normal((B, S, D), dtype=np.float32)
    W = rng.standard_normal((1, D, IK), dtype=np.float32) * 0.1
    out = kernel(x, W)
    print(out.shape, out.dtype)


# revision 46
# speedup vs baseline: 1.0143x; 1.0032x over previous
"""Trainium2 Bass kernel for nn_Caps_Layer (capsule routing layer).

Reference computation (per batch b of 1024):
  u_hat[b] = (x[b] @ W).reshape(512, 5, 5)
  4 rounds of routing:
    c = softmax_over_cap(blog); o = squash(sum_s c*u); blog = einsum(o, u)
  output: o [1024, 5, 5]

Sharding: pure data parallel over batch across 8 cores (128 batches/core).

Per-core design (token-position on SBUF partitions; s = 4p + q):
  - x streamed per 4-batch quad into [128p, 4*4*120] f32 (large contiguous
    DMAs keep the shared HWDGE descriptor generator off the critical path).
  - bf16 transposes via a strided 16-bit view of the f32 data (the high
    half-word of an f32 IS its bf16 truncation): 1 cyc/row on PE, no
    conversion pass, and the PSUM->SBUF xt copy runs in 2x 16-bit mode.
  - bf16 GEMM vs W; matmuls write PSUM with a strided 2-dim out AP so a
    quad lands as (k,i,b4,q4) -- the quad scatter to u2 (k,i,b,q) then has
    (b,q) contiguous and collapses to a 3-dim AP (ACT-legal, one copy).
  - Routing on-chip in fp16, layout (k,i,b,q): sum_s via PE ones-matmul,
    uo product per-chunk on GPSIMD (otherwise idle), k/i reductions as
    paired plane adds, r = 1/||o_raw|| via an int16-bit-trick rsqrt seed
    through ACT Exp (keeps a single activation-table set; a DVE Newton
    step refines the last iteration's r, which scales the output).
  - UNEVEN batch groups [48, 36, 24, 12, 8] pipeline phase1 vs routing:
    later groups (whose u_hat is DMA-gated until late) have shorter
    routing chains, balancing all groups' finish times.
"""

import numpy as np

NCORES = 8
B, S, D = 1024, 512, 120
NCAP, DCAP = 5, 5
IK = NCAP * DCAP  # 25
BC = B // NCORES  # 128 batches per core
TOK = BC * S
ROUTINGS = 4
QB = 4              # s-phases per partition (s = 4p + q)

GSIZES = [48, 36, 24, 12, 8]    # batches per group (sum = BC)
GCHUNK = [16, 18, 12, 12, 8]    # routing psum chunk (<=20 => <=1 psum bank)
NG = len(GSIZES)

N_XT_DVE = 2  # of every 8 xt copies, how many go to DVE (rest ACT)


CURRENT_LABEL = [""]


class _Geo:
    """Per-group layout geometry. u2/cu free dims (k5, i5, bG, q4)."""

    def __init__(self, g):
        self.bg = GSIZES[g]                 # batches in group
        self.b0 = sum(GSIZES[:g])           # first batch
        self.chunk = GCHUNK[g]
        self.nchunk = self.bg // self.chunk
        self.nquad = self.bg // 4
        self.q0 = self.b0 // 4              # first quad (absolute)
        self.k_str = NCAP * self.bg * QB
        self.i_str = self.bg * QB
        self.fg = DCAP * self.k_str
        self.ok = NCAP * self.bg            # o2 k stride
        self.oi = self.bg                   # o2 i stride
        self.pl = NCAP * self.bg            # (i,b) plane
        self.blf = NCAP * self.bg * QB      # (i,b,q) logits
        self.zf = self.bg * QB              # (b,q)


def _build(n_routing=ROUTINGS):
    import math

    import concourse.bass as bass
    import concourse.bacc as bacc
    import concourse.tile as tile
    from concourse import mybir
    from concourse.masks import make_identity

    f32 = mybir.dt.float32
    f16 = mybir.dt.float16
    bf16 = mybir.dt.bfloat16
    i16 = mybir.dt.int16
    AF = mybir.ActivationFunctionType
    # rsqrt-from-f16-bits: int16 view of positive f16 x is affine in
    # log2(x), so r0 = exp(S*bits + C0) ~= x**-0.5 (max rel err ~1.5%);
    # Exp is in the already-loaded ACT table set -> no table switches.
    RS_S = -0.5 * math.log(2.0) / 1024.0
    RS_C0 = 0.5 * math.log(2.0) * (15.0 - 0.043)

    geos = [_Geo(g) for g in range(NG)]

    nc = bacc.Bacc("TRN2", target_bir_lowering=False, debug=False)
    x_d = nc.dram_tensor("x", [TOK, D], f32, kind="ExternalInput")
    w_d = nc.dram_tensor("w", [D, IK], f32, kind="ExternalInput")
    out_d = nc.dram_tensor("out", [1, BC * IK], f32, kind="ExternalOutput")

    # HBM elem(quad c; f, p, q, d) = c*4*61440 + f*61440 + p*480 + q*120 + d
    xr = x_d[:, :]

    def xv4(c):
        return bass.AP(
            tensor=xr.tensor,
            offset=xr.offset + c * 4 * 128 * QB * D,
            ap=[[QB * D, 128], [128 * QB * D, 4], [1, QB * D]],
        )

    def ap_of(tile_ap, free_dims, extra_off=0):
        return bass.AP(
            tensor=tile_ap.tensor,
            offset=tile_ap.offset + extra_off,
            ap=[list(tile_ap.ap[0])] + [list(d) for d in free_dims],
        )

    def row0(tile_ap, free_dims, extra_off=0):
        p0 = [list(tile_ap.ap[0])[0], 1]
        return bass.AP(
            tensor=tile_ap.tensor,
            offset=tile_ap.offset + extra_off,
            ap=[p0] + [list(d) for d in free_dims],
        )

    with tile.TileContext(nc) as tc:
        with (
            tc.tile_pool(name="const", bufs=1) as const,
            tc.tile_pool(name="big", bufs=1) as big,
            tc.tile_pool(name="xin", bufs=4) as xin,
            tc.tile_pool(name="xtsb", bufs=4) as xtsb,
            tc.tile_pool(name="xtps", bufs=2, space="PSUM") as xtps,
            tc.tile_pool(name="ups", bufs=3, space="PSUM") as ups,
            tc.tile_pool(name="ops", bufs=3, space="PSUM") as ops_pool,
        ):
            # ---- constants ----
            w_sb = const.tile([128, IK], f32)
            nc.sync.dma_start(out=w_sb[:D, :], in_=w_d[:, :])
            w16 = const.tile([128, IK], bf16)
            nc.vector.tensor_copy(out=w16[:D, :], in_=w_sb[:D, :])
            ident16 = const.tile([128, 128], bf16)
            make_identity(nc, ident16[:])
            ones16 = const.tile([128, 128], f16)
            nc.vector.memset(ones16[:], 1.0)
            c0t = const.tile([128, 1], f32)
            nc.vector.memset(c0t[:], RS_C0)

            # ---- per-group persistent tensors ----
            u2, cu, o2, sq, bl, pb, e_t, c_t = [], [], [], [], [], [], [], []
            zp, z_t, zi, sp, ss, lr, rr, fin = [], [], [], [], [], [], [], []
            for g, geo in enumerate(geos):
                u2.append(big.tile([128, geo.fg], f16, name=f"u2_{g}"))
                cu.append(big.tile([128, geo.fg], f16, name=f"cu_{g}"))
                o2.append(big.tile([128, DCAP * geo.pl], f16, name=f"o2_{g}"))
                sq.append(big.tile([128, DCAP * geo.pl], f16, name=f"sq_{g}"))
                bl.append(big.tile([128, geo.blf], f16, name=f"bl_{g}"))
                pb.append(big.tile([128, 2 * geo.blf], f16, name=f"pb_{g}"))
                e_t.append(big.tile([128, geo.blf], f16, name=f"e_{g}"))
                c_t.append(big.tile([128, geo.blf], f16, name=f"c_{g}"))
                zp.append(big.tile([128, 2 * geo.zf], f16, name=f"zp_{g}"))
                z_t.append(big.tile([128, geo.zf], f16, name=f"z_{g}"))
                zi.append(big.tile([128, geo.zf], f16, name=f"zi_{g}"))
                sp.append(big.tile([128, 2 * geo.pl], f16, name=f"sp_{g}"))
                ss.append(big.tile([128, geo.pl], f16, name=f"ss_{g}"))
                lr.append(big.tile([128, geo.pl], f32, name=f"lr_{g}"))
                rr.append(big.tile([128, geo.pl], f32, name=f"rr_{g}"))
                fin.append(big.tile([1, geo.bg * IK], f32, name=f"fin_{g}"))

            # ================= Phase 1: u_hat GEMM =================
            def phase1_quad(g, gl):
                CURRENT_LABEL[0] = f"p1.g{g}"
                geo = geos[g]
                if True:
                    # psum quad laid out (k5, i5, b4, q4): strides 80,16,4,1
                    u_ps = ups.tile([128, 16 * IK], f32, name="u_ps")
                    x_sb = xin.tile([128, 4 * QB * D], f32, name="x_sb")
                    nc.sync.dma_start(out=x_sb[:], in_=xv4(geo.q0 + gl))
                    a16 = x_sb[:].bitcast(bf16)
                    for hh in range(2):
                        xt_ps = xtps.tile([128, 1024], bf16, name="xt_ps")
                        for bb in range(2):
                            f = hh * 2 + bb
                            for q in range(QB):
                                t = bb * QB + q
                                src = bass.AP(
                                    tensor=a16.tensor,
                                    offset=a16.offset
                                    + 2 * (f * QB * D + q * D) + 1,
                                    ap=[list(a16.ap[0]), [2, D]],
                                )
                                nc.tensor.transpose(
                                    xt_ps[:D, t * 128:(t + 1) * 128],
                                    src, ident16[:],
                                )
                        xt_sb = xtsb.tile([128, 1024], bf16, name="xt_sb")
                        if (gl * 2 + hh) % 8 < N_XT_DVE:
                            nc.vector.tensor_copy(
                                out=xt_sb[:D, :], in_=xt_ps[:D, :])
                        else:
                            nc.scalar.copy(out=xt_sb[:D, :], in_=xt_ps[:D, :])
                        for t in range(8):
                            bb_, q_ = t // QB, t % QB
                            bloc = hh * 2 + bb_
                            # out cols (i,k) -> psum (k:80, i:16) + b*4 + q
                            dst = ap_of(u_ps[:], [[16, NCAP], [80, DCAP]],
                                        extra_off=bloc * 4 + q_)
                            nc.tensor.matmul(
                                dst,
                                xt_sb[:D, t * 128:(t + 1) * 128],
                                w16[:D, :],
                                start=True, stop=True,
                            )
                    # quad scatter (one 3-dim copy): psum (k,i,b4,q4) ->
                    # u2 (k,i,b,q); (b4,q4) is a contiguous run of 16 both
                    # sides.
                    src = ap_of(u_ps[:], [[80, DCAP], [16, NCAP], [1, 16]])
                    dst = ap_of(u2[g][:], [[geo.k_str, DCAP],
                                           [geo.i_str, NCAP], [1, 16]],
                                extra_off=gl * 4 * QB)
                    nc.scalar.copy(out=dst, in_=src)

            # ================= Phase 2: routing =================
            # Issue stages batched across groups so no engine's in-order
            # queue head-of-line blocks on another group's dependency.
            def routing_stage_a(g, it):
                CURRENT_LABEL[0] = f"a.g{g}.i{it}"
                geo = geos[g]
                u2g, cug, o2g = u2[g][:], cu[g][:], o2[g][:]
                last = it == n_routing - 1
                src_t = u2g if it == 0 else cug
                ck = geo.chunk
                for ci in range(geo.nchunk):
                    o_ps = ops_pool.tile([128, ck * IK], f32, name="o_ps")
                    for q in range(QB):
                        rhs = ap_of(
                            src_t,
                            [[QB, ck], [geo.i_str, NCAP], [geo.k_str, DCAP]],
                            extra_off=q + ci * ck * QB,
                        )
                        nc.tensor.matmul(
                            o_ps[:], ones16[:], rhs,
                            start=(q == 0), stop=(q == QB - 1),
                        )
                    # psum (b,i,k) -> o2 (k,i,b), cast f16
                    dst = ap_of(
                        o2g, [[1, ck], [geo.oi, NCAP], [geo.ok, DCAP]],
                        extra_off=ci * ck,
                    )
                    nc.scalar.copy(out=dst, in_=o_ps[:])
                    if not last:
                        # uo chunk on GPSIMD: cu = u2 * o2_bcast(q)
                        u2c = ap_of(u2g, [[geo.k_str, DCAP],
                                          [geo.i_str, NCAP], [1, ck * QB]],
                                    extra_off=ci * ck * QB)
                        cuc = ap_of(cug, [[geo.k_str, DCAP],
                                          [geo.i_str, NCAP], [1, ck * QB]],
                                    extra_off=ci * ck * QB)
                        o2_bc = ap_of(o2g, [[geo.ok, DCAP], [geo.oi, NCAP],
                                            [1, ck], [0, QB]],
                                      extra_off=ci * ck)
                        nc.gpsimd.tensor_mul(cuc, u2c, o2_bc)

            def routing_stats(g, it):
                CURRENT_LABEL[0] = f"s.g{g}.i{it}"
                geo = geos[g]
                o2g, sqg = o2[g][:], sq[g][:]
                last = it == n_routing - 1
                pl = geo.pl
                # squash stats: ss = sum_k o^2 -> rr = 1/sqrt(ss)
                spg = sp[g][:]
                nc.vector.tensor_mul(sqg, o2g, o2g)
                nc.vector.tensor_add(
                    spg,
                    ap_of(sqg, [[2 * pl, 2], [1, pl]]),
                    ap_of(sqg, [[2 * pl, 2], [1, pl]], extra_off=pl))
                nc.vector.tensor_add(ss[g][:], sp[g][:, :pl], sp[g][:, pl:])
                nc.vector.tensor_add(
                    ss[g][:], ss[g][:],
                    ap_of(sqg, [[1, pl]], extra_off=4 * pl))
                nc.scalar.activation(
                    out=rr[g][:], in_=ss[g][:].bitcast(i16), func=AF.Exp,
                    scale=RS_S, bias=c0t[:])
                if last:
                    # one Newton step: r *= 1.5 - 0.5*ss*r^2 (max err 6e-4);
                    # only the last iteration's r scales the output directly
                    nt = lr[g][:]
                    nc.vector.tensor_mul(nt, rr[g][:], rr[g][:])
                    nc.vector.tensor_mul(nt, nt, ss[g][:])
                    nc.vector.tensor_scalar(
                        out=nt, in0=nt, scalar1=-0.5, scalar2=1.5,
                        op0=mybir.AluOpType.mult, op1=mybir.AluOpType.add)
                    nc.vector.tensor_mul(rr[g][:], rr[g][:], nt)

            def routing_stage_b(g, it):
                CURRENT_LABEL[0] = f"b.g{g}.i{it}"
                geo = geos[g]
                u2g, cug, o2g = u2[g][:], cu[g][:], o2[g][:]
                last = it == n_routing - 1
                blf, zf, pl = geo.blf, geo.zf, geo.pl
                if not last:
                    # blog = sum_k uo (k-plane pair adds), fold r, softmax(i)
                    blg, pbg = bl[g][:], pb[g][:]
                    ks = geo.k_str
                    nc.vector.tensor_add(
                        pbg,
                        ap_of(cug, [[2 * ks, 2], [1, blf]]),
                        ap_of(cug, [[2 * ks, 2], [1, blf]], extra_off=ks))
                    nc.vector.tensor_add(blg, pb[g][:, :blf], pb[g][:, blf:])
                    nc.vector.tensor_add(
                        blg, blg, ap_of(cug, [[1, blf]], extra_off=4 * ks))
                    # r broadcast (i,b) -> (i,b,q): strided, 1x
                    r32_bc = ap_of(rr[g][:], [[geo.oi, NCAP], [1, geo.bg],
                                              [0, QB]])
                    nc.vector.tensor_mul(blg, blg, r32_bc)
                    nc.scalar.activation(out=e_t[g][:], in_=blg, func=AF.Exp)
                    # z = sum_i e  (i-planes of (i,b,q))
                    nc.vector.tensor_add(
                        zp[g][:],
                        ap_of(e_t[g][:], [[2 * zf, 2], [1, zf]]),
                        ap_of(e_t[g][:], [[2 * zf, 2], [1, zf]],
                              extra_off=zf))
                    nc.vector.tensor_add(z_t[g][:], zp[g][:, :zf],
                                         zp[g][:, zf:])
                    nc.vector.tensor_add(
                        z_t[g][:], z_t[g][:],
                        ap_of(e_t[g][:], [[1, zf]], extra_off=4 * zf))
                    with nc.allow_low_precision("softmax denom fp16 ok"):
                        nc.vector.reciprocal(zi[g][:], z_t[g][:])
                    zi_bc = ap_of(zi[g][:], [[0, NCAP], [1, zf]])
                    nc.vector.tensor_mul(c_t[g][:], e_t[g][:], zi_bc)
                    # cu = u2 * c_bcast(k), per chunk so the next
                    # iteration's first matmul can start early
                    ck = geo.chunk
                    for ci in range(geo.nchunk):
                        u2c = ap_of(u2g, [[geo.k_str, DCAP],
                                          [geo.i_str, NCAP], [1, ck * QB]],
                                    extra_off=ci * ck * QB)
                        cuc = ap_of(cug, [[geo.k_str, DCAP],
                                          [geo.i_str, NCAP], [1, ck * QB]],
                                    extra_off=ci * ck * QB)
                        c_bc = ap_of(c_t[g][:],
                                     [[0, DCAP], [geo.i_str, NCAP],
                                      [1, ck * QB]],
                                     extra_off=ci * ck * QB)
                        nc.vector.tensor_mul(cuc, u2c, c_bc)
                else:
                    # fin[(b,i,k)] = o2[(k,i,b)] * r  (row 0; all rows equal)
                    o2_row = row0(o2g, [[1, geo.bg], [geo.oi, NCAP],
                                        [geo.ok, DCAP]])
                    r_row = row0(rr[g][:], [[1, geo.bg], [geo.bg, NCAP],
                                            [0, DCAP]])
                    nc.gpsimd.tensor_mul(fin[g][:], o2_row, r_row)
                    nc.sync.dma_start(
                        out=out_d[:, geo.b0 * IK:(geo.b0 + geo.bg) * IK],
                        in_=fin[g][:],
                    )

            # Feasibility-ordered global issue: each engine's in-order queue
            # then approximates the true dependency order, minimizing
            # head-of-line blocking. Keys are rough start-time estimates
            # (us): DMA delivers ~0.683us/batch; a routing iteration's
            # serial chain is ~(chain_a + chain_b) us.
            units = []
            for g, geo in enumerate(geos):
                for gl in range(geo.nquad):
                    units.append((0.683 * 4 * (geo.q0 + gl), 0,
                                  ("p1", g, gl)))
                p1_end = 0.683 * (geo.b0 + geo.bg) + 1.5
                chain_a = 1.5 + 0.21 * geo.bg   # matmul+o2+uo
                chain_b = 4.0 + 0.09 * geo.bg   # blog+softmax+cu
                tkey = p1_end
                for it in range(n_routing):
                    units.append((tkey, 1, ("a", g, it)))
                    units.append((tkey + 1.5, 2, ("s", g, it)))
                    units.append((tkey + chain_a, 3, ("b", g, it)))
                    tkey += chain_a + chain_b
            units.sort(key=lambda u: (u[0], u[1]))
            for _, _, (kind, g, x) in units:
                if kind == "p1":
                    phase1_quad(g, x)
                elif kind == "a":
                    routing_stage_a(g, x)
                elif kind == "s":
                    routing_stats(g, x)
                else:
                    routing_stage_b(g, x)
    nc.compile()
    return nc


_NC = None


def kernel(x: np.ndarray, W: np.ndarray) -> np.ndarray:
    from concourse.bass_utils import run_bass_kernel_spmd

    global _NC
    if _NC is None:
        _NC = _build()

    x = np.ascontiguousarray(x, dtype=np.float32)
    w = np.ascontiguousarray(W.reshape(D, IK), dtype=np.float32)
    xs = x.reshape(NCORES, TOK, D)
    in_maps = [{"x": xs[i], "w": w} for i in range(NCORES)]
    res = run_bass_kernel_spmd(_NC, in_maps, core_ids=list(range(NCORES)))
    out = np.concatenate(
        [r["out"].reshape(BC, NCAP, DCAP) for r in res.results], axis=0
    )
    return out


if __name__ == "__main__":
    rng = np.random.default_rng(0)
    x = rng.standard_normal((B, S, D), dtype=np.float32)
    W = rng.standard_normal((1, D, IK), dtype=np.float32) * 0.1
    out = kernel(x, W)
    print(out.shape, out.dtype)


# revision 50
# speedup vs baseline: 1.0154x; 1.0011x over previous
"""Trainium2 Bass kernel for nn_Caps_Layer (capsule routing layer).

Reference computation (per batch b of 1024):
  u_hat[b] = (x[b] @ W).reshape(512, 5, 5)
  4 rounds of routing:
    c = softmax_over_cap(blog); o = squash(sum_s c*u); blog = einsum(o, u)
  output: o [1024, 5, 5]

Sharding: pure data parallel over batch across 8 cores (128 batches/core).

Per-core design (token-position on SBUF partitions; s = 4p + q):
  - x streamed per 4-batch quad into [128p, 4*4*120] f32 (large contiguous
    DMAs keep the shared HWDGE descriptor generator off the critical path).
  - bf16 transposes via a strided 16-bit view of the f32 data (the high
    half-word of an f32 IS its bf16 truncation): 1 cyc/row on PE, no
    conversion pass, and the PSUM->SBUF xt copy runs in 2x 16-bit mode.
  - bf16 GEMM vs W; matmuls write PSUM with a strided 2-dim out AP so a
    quad lands as (k,i,b4,q4) -- the quad scatter to u2 (k,i,b,q) then has
    (b,q) contiguous and collapses to a 3-dim AP (ACT-legal, one copy).
  - Routing on-chip in fp16, layout (k,i,b,q): sum_s via PE ones-matmul,
    uo product per-chunk on GPSIMD (otherwise idle), k/i reductions as
    paired plane adds, r = 1/||o_raw|| via an int16-bit-trick rsqrt seed
    through ACT Exp (keeps a single activation-table set; a DVE Newton
    step refines the last iteration's r, which scales the output).
  - UNEVEN batch groups [48, 36, 24, 12, 8] pipeline phase1 vs routing:
    later groups (whose u_hat is DMA-gated until late) have shorter
    routing chains, balancing all groups' finish times.
"""

import numpy as np

NCORES = 8
B, S, D = 1024, 512, 120
NCAP, DCAP = 5, 5
IK = NCAP * DCAP  # 25
BC = B // NCORES  # 128 batches per core
TOK = BC * S
ROUTINGS = 4
QB = 4              # s-phases per partition (s = 4p + q)

GSIZES = [48, 36, 24, 12, 8]    # batches per group (sum = BC)
GCHUNK = [16, 18, 12, 12, 8]    # routing psum chunk (<=20 => <=1 psum bank)
NG = len(GSIZES)

N_XT_DVE = 2  # of every 8 xt copies, how many go to DVE (rest ACT)


CURRENT_LABEL = [""]


class _Geo:
    """Per-group layout geometry. u2/cu free dims (k5, i5, bG, q4)."""

    def __init__(self, g):
        self.bg = GSIZES[g]                 # batches in group
        self.b0 = sum(GSIZES[:g])           # first batch
        self.chunk = GCHUNK[g]
        self.nchunk = self.bg // self.chunk
        self.nquad = self.bg // 4
        self.q0 = self.b0 // 4              # first quad (absolute)
        self.k_str = NCAP * self.bg * QB
        self.i_str = self.bg * QB
        self.fg = DCAP * self.k_str
        self.ok = NCAP * self.bg            # o2 k stride
        self.oi = self.bg                   # o2 i stride
        self.pl = NCAP * self.bg            # (i,b) plane
        self.blf = NCAP * self.bg * QB      # (i,b,q) logits
        self.zf = self.bg * QB              # (b,q)


def _build(n_routing=ROUTINGS):
    import math

    import concourse.bass as bass
    import concourse.bacc as bacc
    import concourse.tile as tile
    from concourse import mybir
    from concourse.masks import make_identity

    f32 = mybir.dt.float32
    f16 = mybir.dt.float16
    bf16 = mybir.dt.bfloat16
    i16 = mybir.dt.int16
    AF = mybir.ActivationFunctionType
    # rsqrt-from-f16-bits: int16 view of positive f16 x is affine in
    # log2(x), so r0 = exp(S*bits + C0) ~= x**-0.5 (max rel err ~1.5%);
    # Exp is in the already-loaded ACT table set -> no table switches.
    RS_S = -0.5 * math.log(2.0) / 1024.0
    RS_C0 = 0.5 * math.log(2.0) * (15.0 - 0.043)

    geos = [_Geo(g) for g in range(NG)]

    nc = bacc.Bacc("TRN2", target_bir_lowering=False, debug=False)
    x_d = nc.dram_tensor("x", [TOK, D], f32, kind="ExternalInput")
    w_d = nc.dram_tensor("w", [D, IK], f32, kind="ExternalInput")
    out_d = nc.dram_tensor("out", [1, BC * IK], f32, kind="ExternalOutput")

    # HBM elem(quad c; f, p, q, d) = c*4*61440 + f*61440 + p*480 + q*120 + d
    xr = x_d[:, :]

    def xv4(c):
        return bass.AP(
            tensor=xr.tensor,
            offset=xr.offset + c * 4 * 128 * QB * D,
            ap=[[QB * D, 128], [128 * QB * D, 4], [1, QB * D]],
        )

    def ap_of(tile_ap, free_dims, extra_off=0):
        return bass.AP(
            tensor=tile_ap.tensor,
            offset=tile_ap.offset + extra_off,
            ap=[list(tile_ap.ap[0])] + [list(d) for d in free_dims],
        )

    def row0(tile_ap, free_dims, extra_off=0):
        p0 = [list(tile_ap.ap[0])[0], 1]
        return bass.AP(
            tensor=tile_ap.tensor,
            offset=tile_ap.offset + extra_off,
            ap=[p0] + [list(d) for d in free_dims],
        )

    with tile.TileContext(nc) as tc:
        with (
            tc.tile_pool(name="const", bufs=1) as const,
            tc.tile_pool(name="big", bufs=1) as big,
            tc.tile_pool(name="xin", bufs=4) as xin,
            tc.tile_pool(name="xtsb", bufs=4) as xtsb,
            tc.tile_pool(name="xtps", bufs=3, space="PSUM") as xtps,
            tc.tile_pool(name="ups", bufs=3, space="PSUM") as ups,
            tc.tile_pool(name="ops", bufs=2, space="PSUM") as ops_pool,
        ):
            # ---- constants ----
            w_sb = const.tile([128, IK], f32)
            nc.sync.dma_start(out=w_sb[:D, :], in_=w_d[:, :])
            w16 = const.tile([128, IK], bf16)
            nc.vector.tensor_copy(out=w16[:D, :], in_=w_sb[:D, :])
            ident16 = const.tile([128, 128], bf16)
            make_identity(nc, ident16[:])
            ones16 = const.tile([128, 128], f16)
            nc.vector.memset(ones16[:], 1.0)
            c0t = const.tile([128, 1], f32)
            nc.vector.memset(c0t[:], RS_C0)

            # ---- per-group persistent tensors ----
            u2, cu, o2, sq, bl, pb, e_t, c_t = [], [], [], [], [], [], [], []
            zp, z_t, zi, sp, ss, lr, rr, fin = [], [], [], [], [], [], [], []
            for g, geo in enumerate(geos):
                u2.append(big.tile([128, geo.fg], f16, name=f"u2_{g}"))
                cu.append(big.tile([128, geo.fg], f16, name=f"cu_{g}"))
                o2.append(big.tile([128, DCAP * geo.pl], f16, name=f"o2_{g}"))
                sq.append(big.tile([128, DCAP * geo.pl], f16, name=f"sq_{g}"))
                bl.append(big.tile([128, geo.blf], f16, name=f"bl_{g}"))
                pb.append(big.tile([128, 2 * geo.blf], f16, name=f"pb_{g}"))
                e_t.append(big.tile([128, geo.blf], f16, name=f"e_{g}"))
                c_t.append(big.tile([128, geo.blf], f16, name=f"c_{g}"))
                zp.append(big.tile([128, 2 * geo.zf], f16, name=f"zp_{g}"))
                z_t.append(big.tile([128, geo.zf], f16, name=f"z_{g}"))
                zi.append(big.tile([128, geo.zf], f16, name=f"zi_{g}"))
                sp.append(big.tile([128, 2 * geo.pl], f16, name=f"sp_{g}"))
                ss.append(big.tile([128, geo.pl], f16, name=f"ss_{g}"))
                lr.append(big.tile([128, geo.pl], f32, name=f"lr_{g}"))
                rr.append(big.tile([128, geo.pl], f32, name=f"rr_{g}"))
                fin.append(big.tile([1, geo.bg * IK], f32, name=f"fin_{g}"))

            # ================= Phase 1: u_hat GEMM =================
            def phase1_quad(g, gl):
                CURRENT_LABEL[0] = f"p1.g{g}"
                geo = geos[g]
                if True:
                    # psum quad laid out (k5, i5, b4, q4): strides 80,16,4,1
                    u_ps = ups.tile([128, 16 * IK], f32, name="u_ps")
                    x_sb = xin.tile([128, 4 * QB * D], f32, name="x_sb")
                    nc.sync.dma_start(out=x_sb[:], in_=xv4(geo.q0 + gl))
                    a16 = x_sb[:].bitcast(bf16)
                    for hh in range(2):
                        xt_ps = xtps.tile([128, 1024], bf16, name="xt_ps")
                        for bb in range(2):
                            f = hh * 2 + bb
                            for q in range(QB):
                                t = bb * QB + q
                                src = bass.AP(
                                    tensor=a16.tensor,
                                    offset=a16.offset
                                    + 2 * (f * QB * D + q * D) + 1,
                                    ap=[list(a16.ap[0]), [2, D]],
                                )
                                nc.tensor.transpose(
                                    xt_ps[:D, t * 128:(t + 1) * 128],
                                    src, ident16[:],
                                )
                        xt_sb = xtsb.tile([128, 1024], bf16, name="xt_sb")
                        if (gl * 2 + hh) % 8 < N_XT_DVE:
                            nc.vector.tensor_copy(
                                out=xt_sb[:D, :], in_=xt_ps[:D, :])
                        else:
                            nc.scalar.copy(out=xt_sb[:D, :], in_=xt_ps[:D, :])
                        for t in range(8):
                            bb_, q_ = t // QB, t % QB
                            bloc = hh * 2 + bb_
                            # out cols (i,k) -> psum (k:80, i:16) + b*4 + q
                            dst = ap_of(u_ps[:], [[16, NCAP], [80, DCAP]],
                                        extra_off=bloc * 4 + q_)
                            nc.tensor.matmul(
                                dst,
                                xt_sb[:D, t * 128:(t + 1) * 128],
                                w16[:D, :],
                                start=True, stop=True,
                            )
                    # quad scatter (one 3-dim copy): psum (k,i,b4,q4) ->
                    # u2 (k,i,b,q); (b4,q4) is a contiguous run of 16 both
                    # sides.
                    src = ap_of(u_ps[:], [[80, DCAP], [16, NCAP], [1, 16]])
                    dst = ap_of(u2[g][:], [[geo.k_str, DCAP],
                                           [geo.i_str, NCAP], [1, 16]],
                                extra_off=gl * 4 * QB)
                    nc.scalar.copy(out=dst, in_=src)

            # ================= Phase 2: routing =================
            # Issue stages batched across groups so no engine's in-order
            # queue head-of-line blocks on another group's dependency.
            def routing_stage_a(g, it):
                CURRENT_LABEL[0] = f"a.g{g}.i{it}"
                geo = geos[g]
                u2g, cug, o2g = u2[g][:], cu[g][:], o2[g][:]
                last = it == n_routing - 1
                src_t = u2g if it == 0 else cug
                ck = geo.chunk
                for ci in range(geo.nchunk):
                    o_ps = ops_pool.tile([128, ck * IK], f32, name="o_ps")
                    for q in range(QB):
                        rhs = ap_of(
                            src_t,
                            [[QB, ck], [geo.i_str, NCAP], [geo.k_str, DCAP]],
                            extra_off=q + ci * ck * QB,
                        )
                        nc.tensor.matmul(
                            o_ps[:], ones16[:], rhs,
                            start=(q == 0), stop=(q == QB - 1),
                        )
                    # psum (b,i,k) -> o2 (k,i,b), cast f16
                    dst = ap_of(
                        o2g, [[1, ck], [geo.oi, NCAP], [geo.ok, DCAP]],
                        extra_off=ci * ck,
                    )
                    nc.scalar.copy(out=dst, in_=o_ps[:])
                    if not last:
                        # uo chunk on GPSIMD: cu = u2 * o2_bcast(q)
                        u2c = ap_of(u2g, [[geo.k_str, DCAP],
                                          [geo.i_str, NCAP], [1, ck * QB]],
                                    extra_off=ci * ck * QB)
                        cuc = ap_of(cug, [[geo.k_str, DCAP],
                                          [geo.i_str, NCAP], [1, ck * QB]],
                                    extra_off=ci * ck * QB)
                        o2_bc = ap_of(o2g, [[geo.ok, DCAP], [geo.oi, NCAP],
                                            [1, ck], [0, QB]],
                                      extra_off=ci * ck)
                        nc.gpsimd.tensor_mul(cuc, u2c, o2_bc)

            def routing_stats(g, it):
                CURRENT_LABEL[0] = f"s.g{g}.i{it}"
                geo = geos[g]
                o2g, sqg = o2[g][:], sq[g][:]
                last = it == n_routing - 1
                pl = geo.pl
                # squash stats: ss = sum_k o^2 -> rr = 1/sqrt(ss)
                spg = sp[g][:]
                nc.vector.tensor_mul(sqg, o2g, o2g)
                nc.vector.tensor_add(
                    spg,
                    ap_of(sqg, [[2 * pl, 2], [1, pl]]),
                    ap_of(sqg, [[2 * pl, 2], [1, pl]], extra_off=pl))
                nc.vector.tensor_add(ss[g][:], sp[g][:, :pl], sp[g][:, pl:])
                nc.vector.tensor_add(
                    ss[g][:], ss[g][:],
                    ap_of(sqg, [[1, pl]], extra_off=4 * pl))
                nc.scalar.activation(
                    out=rr[g][:], in_=ss[g][:].bitcast(i16), func=AF.Exp,
                    scale=RS_S, bias=c0t[:])
                if last:
                    # one Newton step: r *= 1.5 - 0.5*ss*r^2 (max err 6e-4);
                    # only the last iteration's r scales the output directly
                    nt = lr[g][:]
                    nc.vector.tensor_mul(nt, rr[g][:], rr[g][:])
                    nc.vector.tensor_mul(nt, nt, ss[g][:])
                    nc.vector.tensor_scalar(
                        out=nt, in0=nt, scalar1=-0.5, scalar2=1.5,
                        op0=mybir.AluOpType.mult, op1=mybir.AluOpType.add)
                    nc.vector.tensor_mul(rr[g][:], rr[g][:], nt)

            def routing_stage_b(g, it):
                CURRENT_LABEL[0] = f"b.g{g}.i{it}"
                geo = geos[g]
                u2g, cug, o2g = u2[g][:], cu[g][:], o2[g][:]
                last = it == n_routing - 1
                blf, zf, pl = geo.blf, geo.zf, geo.pl
                if not last:
                    # blog = sum_k uo (k-plane pair adds), fold r, softmax(i)
                    blg, pbg = bl[g][:], pb[g][:]
                    ks = geo.k_str
                    nc.vector.tensor_add(
                        pbg,
                        ap_of(cug, [[2 * ks, 2], [1, blf]]),
                        ap_of(cug, [[2 * ks, 2], [1, blf]], extra_off=ks))
                    nc.vector.tensor_add(blg, pb[g][:, :blf], pb[g][:, blf:])
                    nc.vector.tensor_add(
                        blg, blg, ap_of(cug, [[1, blf]], extra_off=4 * ks))
                    # r broadcast (i,b) -> (i,b,q): strided, 1x
                    r32_bc = ap_of(rr[g][:], [[geo.oi, NCAP], [1, geo.bg],
                                              [0, QB]])
                    nc.vector.tensor_mul(blg, blg, r32_bc)
                    nc.scalar.activation(out=e_t[g][:], in_=blg, func=AF.Exp)
                    # z = sum_i e  (i-planes of (i,b,q))
                    nc.vector.tensor_add(
                        zp[g][:],
                        ap_of(e_t[g][:], [[2 * zf, 2], [1, zf]]),
                        ap_of(e_t[g][:], [[2 * zf, 2], [1, zf]],
                              extra_off=zf))
                    nc.vector.tensor_add(z_t[g][:], zp[g][:, :zf],
                                         zp[g][:, zf:])
                    nc.vector.tensor_add(
                        z_t[g][:], z_t[g][:],
                        ap_of(e_t[g][:], [[1, zf]], extra_off=4 * zf))
                    with nc.allow_low_precision("softmax denom fp16 ok"):
                        nc.vector.reciprocal(zi[g][:], z_t[g][:])
                    zi_bc = ap_of(zi[g][:], [[0, NCAP], [1, zf]])
                    nc.vector.tensor_mul(c_t[g][:], e_t[g][:], zi_bc)
                    # cu = u2 * c_bcast(k), per chunk so the next
                    # iteration's first matmul can start early
                    ck = geo.chunk
                    for ci in range(geo.nchunk):
                        u2c = ap_of(u2g, [[geo.k_str, DCAP],
                                          [geo.i_str, NCAP], [1, ck * QB]],
                                    extra_off=ci * ck * QB)
                        cuc = ap_of(cug, [[geo.k_str, DCAP],
                                          [geo.i_str, NCAP], [1, ck * QB]],
                                    extra_off=ci * ck * QB)
                        c_bc = ap_of(c_t[g][:],
                                     [[0, DCAP], [geo.i_str, NCAP],
                                      [1, ck * QB]],
                                     extra_off=ci * ck * QB)
                        nc.vector.tensor_mul(cuc, u2c, c_bc)
                else:
                    # fin[(b,i,k)] = o2[(k,i,b)] * r  (row 0; all rows equal)
                    o2_row = row0(o2g, [[1, geo.bg], [geo.oi, NCAP],
                                        [geo.ok, DCAP]])
                    r_row = row0(rr[g][:], [[1, geo.bg], [geo.bg, NCAP],
                                            [0, DCAP]])
                    nc.gpsimd.tensor_mul(fin[g][:], o2_row, r_row)
                    nc.sync.dma_start(
                        out=out_d[:, geo.b0 * IK:(geo.b0 + geo.bg) * IK],
                        in_=fin[g][:],
                    )

            # Feasibility-ordered global issue: each engine's in-order queue
            # then approximates the true dependency order, minimizing
            # head-of-line blocking. Keys are rough start-time estimates
            # (us): DMA delivers ~0.683us/batch; a routing iteration's
            # serial chain is ~(chain_a + chain_b) us.
            units = []
            for g, geo in enumerate(geos):
                for gl in range(geo.nquad):
                    units.append((0.683 * 4 * (geo.q0 + gl), 0,
                                  ("p1", g, gl)))
                p1_end = 0.683 * (geo.b0 + geo.bg) + 1.5
                chain_a = 1.5 + 0.21 * geo.bg   # matmul+o2+uo
                chain_b = 4.0 + 0.09 * geo.bg   # blog+softmax+cu
                tkey = p1_end
                for it in range(n_routing):
                    units.append((tkey, 1, ("a", g, it)))
                    units.append((tkey + 1.5, 2, ("s", g, it)))
                    units.append((tkey + chain_a, 3, ("b", g, it)))
                    tkey += chain_a + chain_b
            units.sort(key=lambda u: (u[0], u[1]))
            for _, _, (kind, g, x) in units:
                if kind == "p1":
                    phase1_quad(g, x)
                elif kind == "a":
                    routing_stage_a(g, x)
                elif kind == "s":
                    routing_stats(g, x)
                else:
                    routing_stage_b(g, x)
    nc.compile()
    return nc


_NC = None


def kernel(x: np.ndarray, W: np.ndarray) -> np.ndarray:
    from concourse.bass_utils import run_bass_kernel_spmd

    global _NC
    if _NC is None:
        _NC = _build()

    x = np.ascontiguousarray(x, dtype=np.float32)
    w = np.ascontiguousarray(W.reshape(D, IK), dtype=np.float32)
    xs = x.reshape(NCORES, TOK, D)
    in_maps = [{"x": xs[i], "w": w} for i in range(NCORES)]
    res = run_bass_kernel_spmd(_NC, in_maps, core_ids=list(range(NCORES)))
    out = np.concatenate(
        [r["out"].reshape(BC, NCAP, DCAP) for r in res.results], axis=0
    )
    return out


if __name__ == "__main__":
    rng = np.random.default_rng(0)
    x = rng.standard_normal((B, S, D), dtype=np.float32)
    W = rng.standard_normal((1, D, IK), dtype=np.float32) * 0.1
    out = kernel(x, W)
    print(out.shape, out.dtype)


# revision 53
# speedup vs baseline: 1.0294x; 1.0137x over previous
"""Trainium2 Bass kernel for nn_Caps_Layer (capsule routing layer).

Reference computation (per batch b of 1024):
  u_hat[b] = (x[b] @ W).reshape(512, 5, 5)
  4 rounds of routing:
    c = softmax_over_cap(blog); o = squash(sum_s c*u); blog = einsum(o, u)
  output: o [1024, 5, 5]

Sharding: pure data parallel over batch across 8 cores (128 batches/core).

Per-core design (token-position on SBUF partitions; s = 4p + q):
  - x streamed per 4-batch quad into [128p, 4*4*120] f32 (large contiguous
    DMAs keep the shared HWDGE descriptor generator off the critical path).
  - bf16 transposes via a strided 16-bit view of the f32 data (the high
    half-word of an f32 IS its bf16 truncation): 1 cyc/row on PE, no
    conversion pass, and the PSUM->SBUF xt copy runs in 2x 16-bit mode.
  - bf16 GEMM vs W; matmuls write PSUM with a strided 2-dim out AP so a
    quad lands as (k,i,b4,q4) -- the quad scatter to u2 (k,i,b,q) then has
    (b,q) contiguous and collapses to a 3-dim AP (ACT-legal, one copy).
  - Routing on-chip in fp16, layout (k,i,b,q): sum_s via PE ones-matmul,
    uo product per-chunk on GPSIMD (otherwise idle), k/i reductions as
    paired plane adds, r = 1/||o_raw|| via an int16-bit-trick rsqrt seed
    through ACT Exp (keeps a single activation-table set; a DVE Newton
    step refines the last iteration's r, which scales the output).
  - UNEVEN batch groups [48, 36, 24, 12, 8] pipeline phase1 vs routing:
    later groups (whose u_hat is DMA-gated until late) have shorter
    routing chains, balancing all groups' finish times.
"""

import numpy as np

NCORES = 8
B, S, D = 1024, 512, 120
NCAP, DCAP = 5, 5
IK = NCAP * DCAP  # 25
BC = B // NCORES  # 128 batches per core
TOK = BC * S
ROUTINGS = 4
QB = 4              # s-phases per partition (s = 4p + q)

GSIZES = [48, 36, 24, 12, 8]    # batches per group (sum = BC)
GCHUNK = [16, 18, 12, 12, 8]    # routing psum chunk (<=20 => <=1 psum bank)
NG = len(GSIZES)

N_XT_DVE = 2  # of every 8 xt copies, how many go to DVE (rest ACT)


CURRENT_LABEL = [""]


class _Geo:
    """Per-group layout geometry. u2/cu free dims (k5, i5, bG, q4)."""

    def __init__(self, g):
        self.bg = GSIZES[g]                 # batches in group
        self.b0 = sum(GSIZES[:g])           # first batch
        self.chunk = GCHUNK[g]
        self.nchunk = self.bg // self.chunk
        self.nquad = self.bg // 4
        self.q0 = self.b0 // 4              # first quad (absolute)
        self.k_str = NCAP * self.bg * QB
        self.i_str = self.bg * QB
        self.fg = DCAP * self.k_str
        self.ok = NCAP * self.bg            # o2 k stride
        self.oi = self.bg                   # o2 i stride
        self.pl = NCAP * self.bg            # (i,b) plane
        self.blf = NCAP * self.bg * QB      # (i,b,q) logits
        self.zf = self.bg * QB              # (b,q)


def _build(n_routing=ROUTINGS):
    import math

    import concourse.bass as bass
    import concourse.bacc as bacc
    import concourse.tile as tile
    from concourse import mybir
    from concourse.masks import make_identity

    f32 = mybir.dt.float32
    f16 = mybir.dt.float16
    bf16 = mybir.dt.bfloat16
    i16 = mybir.dt.int16
    AF = mybir.ActivationFunctionType
    # rsqrt-from-f16-bits: int16 view of positive f16 x is affine in
    # log2(x), so r0 = exp(S*bits + C0) ~= x**-0.5 (max rel err ~1.5%);
    # Exp is in the already-loaded ACT table set -> no table switches.
    RS_S = -0.5 * math.log(2.0) / 1024.0
    RS_C0 = 0.5 * math.log(2.0) * (15.0 - 0.043)

    geos = [_Geo(g) for g in range(NG)]

    nc = bacc.Bacc("TRN2", target_bir_lowering=False, debug=False)
    x_d = nc.dram_tensor("x", [TOK, D], f32, kind="ExternalInput")
    w_d = nc.dram_tensor("w", [D, IK], f32, kind="ExternalInput")
    out_d = nc.dram_tensor("out", [1, BC * IK], f32, kind="ExternalOutput")

    # HBM elem(quad c; f, p, q, d) = c*4*61440 + f*61440 + p*480 + q*120 + d
    xr = x_d[:, :]

    def xv4(c):
        return bass.AP(
            tensor=xr.tensor,
            offset=xr.offset + c * 4 * 128 * QB * D,
            ap=[[QB * D, 128], [128 * QB * D, 4], [1, QB * D]],
        )

    def ap_of(tile_ap, free_dims, extra_off=0):
        return bass.AP(
            tensor=tile_ap.tensor,
            offset=tile_ap.offset + extra_off,
            ap=[list(tile_ap.ap[0])] + [list(d) for d in free_dims],
        )

    def row0(tile_ap, free_dims, extra_off=0):
        p0 = [list(tile_ap.ap[0])[0], 1]
        return bass.AP(
            tensor=tile_ap.tensor,
            offset=tile_ap.offset + extra_off,
            ap=[p0] + [list(d) for d in free_dims],
        )

    with tile.TileContext(nc) as tc:
        with (
            tc.tile_pool(name="const", bufs=1) as const,
            tc.tile_pool(name="big", bufs=1) as big,
            tc.tile_pool(name="xin", bufs=4) as xin,
            tc.tile_pool(name="xtsb", bufs=4) as xtsb,
            tc.tile_pool(name="xtps", bufs=3, space="PSUM") as xtps,
            tc.tile_pool(name="ups", bufs=3, space="PSUM") as ups,
            tc.tile_pool(name="ops", bufs=2, space="PSUM") as ops_pool,
        ):
            # ---- constants ----
            w_sb = const.tile([128, IK], f32)
            nc.sync.dma_start(out=w_sb[:D, :], in_=w_d[:, :])
            w16 = const.tile([128, IK], bf16)
            nc.vector.tensor_copy(out=w16[:D, :], in_=w_sb[:D, :])
            ident16 = const.tile([128, 128], bf16)
            make_identity(nc, ident16[:])
            ones16 = const.tile([128, 128], f16)
            nc.vector.memset(ones16[:], 1.0)
            c0t = const.tile([128, 1], f32)
            nc.vector.memset(c0t[:], RS_C0)

            # ---- per-group persistent tensors ----
            u2, cu, o2, sq, bl, pb, e_t, c_t = [], [], [], [], [], [], [], []
            zp, z_t, zi, sp, ss, lr, rr, fin = [], [], [], [], [], [], [], []
            for g, geo in enumerate(geos):
                u2.append(big.tile([128, geo.fg], f16, name=f"u2_{g}"))
                cu.append(big.tile([128, geo.fg], f16, name=f"cu_{g}"))
                o2.append(big.tile([128, DCAP * geo.pl], f16, name=f"o2_{g}"))
                sq.append(big.tile([128, DCAP * geo.pl], f16, name=f"sq_{g}"))
                bl.append(big.tile([128, geo.blf], f16, name=f"bl_{g}"))
                pb.append(big.tile([128, 2 * geo.blf], f16, name=f"pb_{g}"))
                e_t.append(big.tile([128, geo.blf], f16, name=f"e_{g}"))
                c_t.append(big.tile([128, geo.blf], f16, name=f"c_{g}"))
                zp.append(big.tile([128, 2 * geo.zf], f16, name=f"zp_{g}"))
                z_t.append(big.tile([128, geo.zf], f16, name=f"z_{g}"))
                zi.append(big.tile([128, geo.zf], f16, name=f"zi_{g}"))
                sp.append(big.tile([128, 2 * geo.pl], f16, name=f"sp_{g}"))
                ss.append(big.tile([128, geo.pl], f16, name=f"ss_{g}"))
                lr.append(big.tile([128, geo.pl], f32, name=f"lr_{g}"))
                rr.append(big.tile([128, geo.pl], f32, name=f"rr_{g}"))
                fin.append(big.tile([1, geo.bg * IK], f32, name=f"fin_{g}"))

            # ================= Phase 1: u_hat GEMM =================
            def phase1_quad(g, gl):
                CURRENT_LABEL[0] = f"p1.g{g}"
                geo = geos[g]
                if True:
                    # psum quad laid out (k5, i5, b4, q4): strides 80,16,4,1
                    u_ps = ups.tile([128, 16 * IK], f32, name="u_ps")
                    x_sb = xin.tile([128, 4 * QB * D], f32, name="x_sb")
                    nc.sync.dma_start(out=x_sb[:], in_=xv4(geo.q0 + gl))
                    a16 = x_sb[:].bitcast(bf16)
                    for hh in range(2):
                        xt_ps = xtps.tile([128, 1024], bf16, name="xt_ps")
                        for bb in range(2):
                            f = hh * 2 + bb
                            for q in range(QB):
                                t = bb * QB + q
                                src = bass.AP(
                                    tensor=a16.tensor,
                                    offset=a16.offset
                                    + 2 * (f * QB * D + q * D) + 1,
                                    ap=[list(a16.ap[0]), [2, D]],
                                )
                                nc.tensor.transpose(
                                    xt_ps[:D, t * 128:(t + 1) * 128],
                                    src, ident16[:],
                                )
                        xt_sb = xtsb.tile([128, 1024], bf16, name="xt_sb")
                        if (gl * 2 + hh) % 4 == 0:  # 2-of-8, spread
                            nc.vector.tensor_copy(
                                out=xt_sb[:D, :], in_=xt_ps[:D, :])
                        else:
                            nc.scalar.copy(out=xt_sb[:D, :], in_=xt_ps[:D, :])
                        for t in range(8):
                            bb_, q_ = t // QB, t % QB
                            bloc = hh * 2 + bb_
                            # out cols (i,k) -> psum (k:80, i:16) + b*4 + q
                            dst = ap_of(u_ps[:], [[16, NCAP], [80, DCAP]],
                                        extra_off=bloc * 4 + q_)
                            nc.tensor.matmul(
                                dst,
                                xt_sb[:D, t * 128:(t + 1) * 128],
                                w16[:D, :],
                                start=True, stop=True,
                            )
                    # quad scatter (one 3-dim copy): psum (k,i,b4,q4) ->
                    # u2 (k,i,b,q); (b4,q4) is a contiguous run of 16 both
                    # sides.
                    src = ap_of(u_ps[:], [[80, DCAP], [16, NCAP], [1, 16]])
                    dst = ap_of(u2[g][:], [[geo.k_str, DCAP],
                                           [geo.i_str, NCAP], [1, 16]],
                                extra_off=gl * 4 * QB)
                    nc.scalar.copy(out=dst, in_=src)

            # ================= Phase 2: routing =================
            # Issue stages batched across groups so no engine's in-order
            # queue head-of-line blocks on another group's dependency.
            def routing_stage_a(g, it):
                CURRENT_LABEL[0] = f"a.g{g}.i{it}"
                geo = geos[g]
                u2g, cug, o2g = u2[g][:], cu[g][:], o2[g][:]
                last = it == n_routing - 1
                src_t = u2g if it == 0 else cug
                ck = geo.chunk
                for ci in range(geo.nchunk):
                    o_ps = ops_pool.tile([128, ck * IK], f32, name="o_ps")
                    for q in range(QB):
                        rhs = ap_of(
                            src_t,
                            [[QB, ck], [geo.i_str, NCAP], [geo.k_str, DCAP]],
                            extra_off=q + ci * ck * QB,
                        )
                        nc.tensor.matmul(
                            o_ps[:], ones16[:], rhs,
                            start=(q == 0), stop=(q == QB - 1),
                        )
                    # psum (b,i,k) -> o2 (k,i,b), cast f16
                    dst = ap_of(
                        o2g, [[1, ck], [geo.oi, NCAP], [geo.ok, DCAP]],
                        extra_off=ci * ck,
                    )
                    nc.scalar.copy(out=dst, in_=o_ps[:])
                    if not last:
                        # uo chunk on GPSIMD: cu = u2 * o2_bcast(q)
                        u2c = ap_of(u2g, [[geo.k_str, DCAP],
                                          [geo.i_str, NCAP], [1, ck * QB]],
                                    extra_off=ci * ck * QB)
                        cuc = ap_of(cug, [[geo.k_str, DCAP],
                                          [geo.i_str, NCAP], [1, ck * QB]],
                                    extra_off=ci * ck * QB)
                        o2_bc = ap_of(o2g, [[geo.ok, DCAP], [geo.oi, NCAP],
                                            [1, ck], [0, QB]],
                                      extra_off=ci * ck)
                        nc.gpsimd.tensor_mul(cuc, u2c, o2_bc)

            def routing_stats(g, it):
                CURRENT_LABEL[0] = f"s.g{g}.i{it}"
                geo = geos[g]
                o2g, sqg = o2[g][:], sq[g][:]
                last = it == n_routing - 1
                pl = geo.pl
                # squash stats: ss = sum_k o^2 -> rr = 1/sqrt(ss)
                spg = sp[g][:]
                nc.vector.tensor_mul(sqg, o2g, o2g)
                nc.vector.tensor_add(
                    spg,
                    ap_of(sqg, [[2 * pl, 2], [1, pl]]),
                    ap_of(sqg, [[2 * pl, 2], [1, pl]], extra_off=pl))
                nc.vector.tensor_add(ss[g][:], sp[g][:, :pl], sp[g][:, pl:])
                nc.vector.tensor_add(
                    ss[g][:], ss[g][:],
                    ap_of(sqg, [[1, pl]], extra_off=4 * pl))
                nc.scalar.activation(
                    out=rr[g][:], in_=ss[g][:].bitcast(i16), func=AF.Exp,
                    scale=RS_S, bias=c0t[:])
                if last:
                    # one Newton step: r *= 1.5 - 0.5*ss*r^2 (max err 6e-4);
                    # only the last iteration's r scales the output directly
                    nt = lr[g][:]
                    nc.vector.tensor_mul(nt, rr[g][:], rr[g][:])
                    nc.vector.tensor_mul(nt, nt, ss[g][:])
                    nc.vector.tensor_scalar(
                        out=nt, in0=nt, scalar1=-0.5, scalar2=1.5,
                        op0=mybir.AluOpType.mult, op1=mybir.AluOpType.add)
                    nc.vector.tensor_mul(rr[g][:], rr[g][:], nt)

            def routing_stage_b(g, it):
                CURRENT_LABEL[0] = f"b.g{g}.i{it}"
                geo = geos[g]
                u2g, cug, o2g = u2[g][:], cu[g][:], o2[g][:]
                last = it == n_routing - 1
                blf, zf, pl = geo.blf, geo.zf, geo.pl
                if not last:
                    # blog = sum_k uo (k-plane pair adds), fold r, softmax(i)
                    blg, pbg = bl[g][:], pb[g][:]
                    ks = geo.k_str
                    nc.vector.tensor_add(
                        pbg,
                        ap_of(cug, [[2 * ks, 2], [1, blf]]),
                        ap_of(cug, [[2 * ks, 2], [1, blf]], extra_off=ks))
                    nc.vector.tensor_add(blg, pb[g][:, :blf], pb[g][:, blf:])
                    nc.vector.tensor_add(
                        blg, blg, ap_of(cug, [[1, blf]], extra_off=4 * ks))
                    # r broadcast (i,b) -> (i,b,q): strided, 1x
                    r32_bc = ap_of(rr[g][:], [[geo.oi, NCAP], [1, geo.bg],
                                              [0, QB]])
                    nc.vector.tensor_mul(blg, blg, r32_bc)
                    nc.scalar.activation(out=e_t[g][:], in_=blg, func=AF.Exp)
                    # z = sum_i e  (i-planes of (i,b,q))
                    nc.vector.tensor_add(
                        zp[g][:],
                        ap_of(e_t[g][:], [[2 * zf, 2], [1, zf]]),
                        ap_of(e_t[g][:], [[2 * zf, 2], [1, zf]],
                              extra_off=zf))
                    nc.vector.tensor_add(z_t[g][:], zp[g][:, :zf],
                                         zp[g][:, zf:])
                    nc.vector.tensor_add(
                        z_t[g][:], z_t[g][:],
                        ap_of(e_t[g][:], [[1, zf]], extra_off=4 * zf))
                    with nc.allow_low_precision("softmax denom fp16 ok"):
                        nc.vector.reciprocal(zi[g][:], z_t[g][:])
                    zi_bc = ap_of(zi[g][:], [[0, NCAP], [1, zf]])
                    nc.vector.tensor_mul(c_t[g][:], e_t[g][:], zi_bc)
                    # cu = u2 * c_bcast(k), per chunk so the next
                    # iteration's first matmul can start early
                    ck = geo.chunk
                    for ci in range(geo.nchunk):
                        u2c = ap_of(u2g, [[geo.k_str, DCAP],
                                          [geo.i_str, NCAP], [1, ck * QB]],
                                    extra_off=ci * ck * QB)
                        cuc = ap_of(cug, [[geo.k_str, DCAP],
                                          [geo.i_str, NCAP], [1, ck * QB]],
                                    extra_off=ci * ck * QB)
                        c_bc = ap_of(c_t[g][:],
                                     [[0, DCAP], [geo.i_str, NCAP],
                                      [1, ck * QB]],
                                     extra_off=ci * ck * QB)
                        nc.vector.tensor_mul(cuc, u2c, c_bc)
                else:
                    # fin[(b,i,k)] = o2[(k,i,b)] * r  (row 0; all rows equal)
                    o2_row = row0(o2g, [[1, geo.bg], [geo.oi, NCAP],
                                        [geo.ok, DCAP]])
                    r_row = row0(rr[g][:], [[1, geo.bg], [geo.bg, NCAP],
                                            [0, DCAP]])
                    nc.gpsimd.tensor_mul(fin[g][:], o2_row, r_row)
                    nc.sync.dma_start(
                        out=out_d[:, geo.b0 * IK:(geo.b0 + geo.bg) * IK],
                        in_=fin[g][:],
                    )

            # Feasibility-ordered global issue: each engine's in-order queue
            # then approximates the true dependency order, minimizing
            # head-of-line blocking. Keys are rough start-time estimates
            # (us): DMA delivers ~0.683us/batch; a routing iteration's
            # serial chain is ~(chain_a + chain_b) us.
            units = []
            for g, geo in enumerate(geos):
                for gl in range(geo.nquad):
                    units.append((0.683 * 4 * (geo.q0 + gl), 0,
                                  ("p1", g, gl)))
                p1_end = 0.683 * (geo.b0 + geo.bg) + 1.5
                chain_a = 1.5 + 0.21 * geo.bg   # matmul+o2+uo
                chain_b = 4.0 + 0.09 * geo.bg   # blog+softmax+cu
                tkey = p1_end
                for it in range(n_routing):
                    units.append((tkey, 1, ("a", g, it)))
                    units.append((tkey + 1.5, 2, ("s", g, it)))
                    units.append((tkey + chain_a, 3, ("b", g, it)))
                    tkey += chain_a + chain_b
            units.sort(key=lambda u: (u[0], u[1]))
            for _, _, (kind, g, x) in units:
                if kind == "p1":
                    phase1_quad(g, x)
                elif kind == "a":
                    routing_stage_a(g, x)
                elif kind == "s":
                    routing_stats(g, x)
                else:
                    routing_stage_b(g, x)
    nc.compile()
    return nc


_NC = None


def kernel(x: np.ndarray, W: np.ndarray) -> np.ndarray:
    from concourse.bass_utils import run_bass_kernel_spmd

    global _NC
    if _NC is None:
        _NC = _build()

    x = np.ascontiguousarray(x, dtype=np.float32)
    w = np.ascontiguousarray(W.reshape(D, IK), dtype=np.float32)
    xs = x.reshape(NCORES, TOK, D)
    in_maps = [{"x": xs[i], "w": w} for i in range(NCORES)]
    res = run_bass_kernel_spmd(_NC, in_maps, core_ids=list(range(NCORES)))
    out = np.concatenate(
        [r["out"].reshape(BC, NCAP, DCAP) for r in res.results], axis=0
    )
    return out


if __name__ == "__main__":
    rng = np.random.default_rng(0)
    x = rng.standard_normal((B, S, D), dtype=np.float32)
    W = rng.standard_normal((1, D, IK), dtype=np.float32) * 0.1
    out = kernel(x, W)
    print(out.shape, out.dtype)
